# revision 5
# baseline (speedup 1.0000x reference)
"""Seq2seq RNN with attention on 8 TRN2 NeuronCores.

Data-parallel over batch (B=32 -> 4 per core). Structure:

- Attention is hoisted OUT of the decoder loop: the decoder recurrence
  h_t = tanh(h_{t-1}@U + x_t + b) does not depend on attention, so the h
  chain runs alone (5 tiny matmuls + 1 activation per step) and attention
  + context + output-projection are computed per 32-step block as batched
  matmuls, split into small pieces interleaved one-per-step with the
  decoder so the serial softmax chain never head-of-line-blocks the
  engine FIFOs.
- The 2-layer encoder runs layer 2 skewed one step behind layer 1, so
  each step issues ONE activation computing [h1(t), h2(t-1)] — the
  recurrence critical path is a single PE->ACT->PE round trip per step.
- All elementwise adds in the recurrences are folded into PSUM via
  identity matmuls (PE accumulates x_t / h1 / bias on top of U products);
  the DVE is off the critical path entirely.
- b_enc1 / b_dec are folded into the embedding tables on the host.
- W_out.T (d x V, bf16) is DMA-prefetched into SBUF during the encoder,
  chunked and gated behind the embedding gathers so the small setup
  transfers win the DMA queue.
- The final projection (the only big GEMM, 107us of PE time) is emitted
  interleaved with the decoder blocks; 1024-wide PSUM->SBUF logit copies
  alternate between DVE and ACT; output DMAs are 1024 columns wide.
"""

import numpy as np
from collections import deque

import concourse.bass as bass
import concourse.bacc as bacc
import concourse.tile as tile
from concourse import mybir
from concourse.bass_utils import run_bass_kernel_spmd
from concourse.masks import make_identity

D = 256
V = 32000
T = 128  # T_SRC == T_TGT == 128
B = 32
NCORES = 8
BL = B // NCORES  # 4 batch elements per core
KC = D // 128  # 2 d-chunks of 128
TB = 32  # decoder block size
NBLK = T // TB
DT = mybir.dt.float32
BF = mybir.dt.bfloat16
NPBF = mybir.dt.np(BF)
AF = mybir.ActivationFunctionType
ALU = mybir.AluOpType
AX = mybir.AxisListType

_CACHE = {}

# projection column chunks: pairs that share one 1024-wide output DMA
def _nchunks():
    out = []
    n0 = 0
    while n0 < V:
        out.append((n0, min(512, V - n0)))
        n0 += 512
    return out


def _build(interleave=True):
    nc = bacc.Bacc(None)

    u_d = nc.declare_dram_parameter("u", [D, D], BF, isOutput=False)
    cwt_d = nc.declare_dram_parameter("ctx_wt", [D, D], BF, isOutput=False)
    wot_d = nc.declare_dram_parameter("w_out_t", [D, V], BF, isOutput=False)
    een_d = nc.declare_dram_parameter("e_en", [V, D], BF, isOutput=False)
    ede_d = nc.declare_dram_parameter("e_de", [V, D], BF, isOutput=False)
    b2b_d = nc.declare_dram_parameter("b2b", [128, KC * BL], BF, isOutput=False)
    si_d = nc.declare_dram_parameter("src_idx", [T, BL], mybir.dt.int32, isOutput=False)
    ti_d = nc.declare_dram_parameter("tgt_idx", [T, BL], mybir.dt.int32, isOutput=False)
    out_d = nc.declare_dram_parameter("out", [T * BL, V], BF, isOutput=True)

    with tile.TileContext(nc) as tc:
        with (
            tc.tile_pool(name="persist", bufs=1) as pp,
            tc.tile_pool(name="work", bufs=4) as wp,
        ):
            # ---- persistent SBUF tiles ----
            u_sb = pp.tile([128, KC, D], BF, tag="u")
            cwt_sb = pp.tile([128, KC, D], BF, tag="cwt")
            w_sb = pp.tile([128, KC, V], BF, tag="w")  # W_out.T chunks
            ident = pp.tile([128, 128], DT, tag="ident")
            identb = pp.tile([128, 128], BF, tag="identb")
            ones1 = pp.tile([1, TB], BF, tag="ones1")
            b2b_sb = pp.tile([128, KC, BL], BF, tag="b2b")
            si_sb = pp.tile([T, BL], mybir.dt.int32, tag="si")
            ti_sb = pp.tile([T, BL], mybir.dt.int32, tag="ti")
            maddb = pp.tile([1, BL, T], BF, tag="maddb")  # -1e9 at PAD, on part 0
            xs = pp.tile([128, T, KC, BL], BF, tag="xs")  # x_src' [d, t, m, b]
            xt = pp.tile([128, T, KC, BL], BF, tag="xt")  # x_tgt'
            hd_enc = pp.tile([128, KC, BL, T], BF, tag="hd")  # H' [d,k,b,t]
            ht_enc = pp.tile([128, BL, KC, 128], BF, tag="ht")  # H^T [t,b,k,d]
            hcat = pp.tile([128, 2, KC, BL], BF, tag="hcat")  # enc state [layer,m,b]
            hT_sb = pp.tile([128, KC, BL], BF, tag="hT")  # enc final h2
            hd_dec = pp.tile([128, KC, T * BL], BF, tag="hdd")  # dec h', r=t*BL+b
            hd_dbt = pp.tile([128, KC, BL, T], BF, tag="hdbt")  # dec h' [d,k,b,t]
            houts = pp.tile([128, KC, T * BL], BF, tag="houts")  # outs'

            # ---- small constant loads (si/ti first: gathers depend on them;
            # u/cwt/b2b are only needed from encoder step 1, loaded later) ----
            nc.sync.dma_start(out=si_sb[:, :], in_=si_d[:, :])
            nc.sync.dma_start(out=ti_sb[:, :], in_=ti_d[:, :])
            make_identity(nc, ident[:, :])
            nc.vector.tensor_copy(out=identb[:, :], in_=ident[:, :])
            nc.vector.memset(ones1[:, :], 1.0)

            # ---- mask + embedding gathers + transposes ----
            with tc.tile_pool(name="pst", bufs=4, space="PSUM") as pst:
                mf = wp.tile([T, BL], DT, tag="mf")
                nc.vector.tensor_copy(out=mf[:, :], in_=si_sb[:, :])
                m01 = wp.tile([T, BL], DT, tag="m01")
                nc.vector.tensor_scalar(
                    out=m01[:, :], in0=mf[:, :], scalar1=0.0, scalar2=None,
                    op0=ALU.is_equal,
                )
                for b in range(BL):
                    psM = pst.tile([1, T], DT, tag="psM")
                    nc.tensor.matmul(out=psM[:, :], lhsT=m01[:, b:b + 1],
                                     rhs=ident[:, :], start=True, stop=True)
                    nc.vector.tensor_scalar(
                        out=maddb[:, b, :], in0=psM[:, :], scalar1=-1e9,
                        scalar2=None, op0=ALU.mult,
                    )

                for (idx_sb, e_d, xdst) in ((si_sb, een_d, xs), (ti_sb, ede_d, xt)):
                    for b in range(BL):
                        xg = wp.tile([T, D], BF, tag="xg", bufs=8)
                        nc.gpsimd.indirect_dma_start(
                            out=xg[:, :],
                            out_offset=None,
                            in_=e_d[:, :],
                            in_offset=bass.IndirectOffsetOnAxis(
                                ap=idx_sb[:, b:b + 1], axis=0),
                        )
                        for k in range(KC):
                            tp = pst.tile([128, 128], DT, tag="tp")
                            nc.tensor.matmul(
                                out=tp[:, :], lhsT=xg[:, k * 128:(k + 1) * 128],
                                rhs=identb[:, :], start=True, stop=True)
                            nc.vector.tensor_copy(out=xdst[:, :, k, b], in_=tp[:, :])

            for k in range(KC):
                nc.sync.dma_start(out=u_sb[:, k, :], in_=u_d[k * 128:(k + 1) * 128, :])
                nc.sync.dma_start(out=cwt_sb[:, k, :], in_=cwt_d[k * 128:(k + 1) * 128, :])
            nc.sync.dma_start(out=b2b_sb[:, :, :], in_=b2b_d[:, :])

            # ---- big weight prefetch. Tiny gpsimd writes into each chunk's
            # corner (reading the last gather output) force every weight DMA
            # to dispatch after the gathers, so the small transfers win the
            # DMA queue and the encoder starts immediately. ----
            WCH = 4000
            for w0 in range(0, V, WCH):
                for k in range(KC):
                    nc.gpsimd.tensor_copy(out=w_sb[0:1, k, w0:w0 + 4],
                                          in_=xt[0:1, T - 1, 0, 0:4])
            for w0 in range(0, V, WCH):
                for k in range(KC):
                    nc.sync.dma_start(
                        out=w_sb[:, k, w0:w0 + WCH],
                        in_=wot_d[k * 128:(k + 1) * 128, w0:w0 + WCH])

            # ---- encoder: layer 2 runs one step skewed so each step is ONE
            # activation computing [h1(t), h2(t-1)] — halves the ACT SEQ
            # serialization on the recurrence critical path. ----
            nc.vector.memset(hcat[:, 1, :, :], 0.0)  # h2(-1) = 0
            with tc.tile_pool(name="pe1", bufs=2, space="PSUM") as pe1:
                for t in range(T + 1):
                    if t == 0:
                        # h1(0) = tanh(x0) (b1 folded into E)
                        nc.scalar.activation(
                            out=hcat[:, 0, :, :], in_=xs[:, 0, :, :], func=AF.Tanh)
                        continue
                    bk1 = pe1.tile([128, 2, KC, BL], DT, tag="b1")
                    if t < T:
                        for m in range(KC):
                            for k in range(KC):
                                nc.tensor.matmul(
                                    out=bk1[:, 0, m, :],
                                    lhsT=u_sb[:, k, m * 128:(m + 1) * 128],
                                    rhs=hcat[:, 0, k, :],
                                    start=(m == 0 and k == 0), stop=False)
                        nc.tensor.matmul(
                            out=bk1[:, 0, :, :], lhsT=identb[:, :],
                            rhs=xs[:, t, :, :], start=False, stop=False)
                    for m in range(KC):
                        for k in range(KC):
                            nc.tensor.matmul(
                                out=bk1[:, 1, m, :],
                                lhsT=u_sb[:, k, m * 128:(m + 1) * 128],
                                rhs=hcat[:, 1, k, :],
                                start=(t == T and m == 0 and k == 0),
                                stop=False)
                    nc.tensor.matmul(
                        out=bk1[:, 1, :, :], lhsT=identb[:, :],
                        rhs=b2b_sb[:, :, :], start=False, stop=False)
                    nc.tensor.matmul(
                        out=bk1[:, 1, :, :], lhsT=identb[:, :],
                        rhs=hcat[:, 0, :, :], start=False, stop=True)
                    if t < T:
                        nc.scalar.activation(
                            out=hcat[:, :, :, :], in_=bk1[:, :, :, :],
                            func=AF.Tanh)
                    else:
                        nc.scalar.activation(
                            out=hcat[:, 1, :, :], in_=bk1[:, 1, :, :],
                            func=AF.Tanh)
                    nc.gpsimd.tensor_copy(out=hd_enc[:, :, :, t - 1],
                                          in_=hcat[:, 1, :, :])

            nc.gpsimd.tensor_copy(out=hT_sb[:, :, :], in_=hd_enc[:, :, :, T - 1])

            # ---- decoder + attention + projection (block-pipelined) ----
            dchunks = []
            n0 = 0
            while n0 < V:
                dchunks.append((n0, min(1024, V - n0)))
                n0 += 1024
            with (
                tc.tile_pool(name="pd", bufs=1, space="PSUM") as pd,
                tc.tile_pool(name="pa", bufs=1, space="PSUM") as pa,
                tc.tile_pool(name="pl", bufs=2, space="PSUM") as pl,
                tc.tile_pool(name="lt", bufs=4) as ltp,
                tc.tile_pool(name="aw", bufs=2) as awp,
            ):
                pending = deque()
                pieces = deque()

                def make_attention_pieces(j):
                    """Attention for block j as small thunks, interleaved one
                    per decoder step of block j+1 so the serial softmax chain
                    never head-of-line-blocks projection work in the FIFOs."""
                    t0 = j * TB
                    st = {}

                    def p_scores(b):
                        def f():
                            if b == 0:
                                st["psS"] = pa.tile([TB, BL, 128], DT, tag="s",
                                                    name="psS")
                            psS = st["psS"]
                            for k in range(KC):
                                nc.tensor.matmul(
                                    out=psS[:, b, :],
                                    lhsT=hd_dbt[:, k, b, t0:t0 + TB],
                                    rhs=hd_enc[:, k, b, :],
                                    start=(b == 0 and k == 0), stop=False)
                            nc.tensor.matmul(
                                out=psS[:, b, :], lhsT=ones1[:, :],
                                rhs=maddb[:, b, :],
                                start=False, stop=(b == BL - 1))
                        return f

                    def p_exp():
                        ex = awp.tile([TB, BL, 128], DT, tag="ex")
                        st["ex"] = ex
                        nc.scalar.activation(out=ex[:, :, :],
                                             in_=st["psS"][:, :, :],
                                             func=AF.Exp, scale=1.0 / 16.0)
                        st["alpha"] = awp.tile([TB, BL, 128], BF, tag="alpha",
                                               name="alpha")

                    def p_soft(b):
                        def f():
                            sm = wp.tile([TB, 1], DT, tag="sm")
                            nc.vector.reduce_sum(out=sm[:, :],
                                                 in_=st["ex"][:, b, :],
                                                 axis=AX.X)
                            rs = wp.tile([TB, 1], DT, tag="rs")
                            nc.vector.reciprocal(out=rs[:, :], in_=sm[:, :])
                            nc.vector.tensor_scalar(
                                out=st["alpha"][:, b, :], in0=st["ex"][:, b, :],
                                scalar1=rs[:, :1], scalar2=None, op0=ALU.mult)
                        return f

                    def p_aT(b):
                        def f():
                            if b == 0:
                                st["aT"] = awp.tile([128, BL, TB], BF,
                                                    tag="aT", name="aT")
                            psT = pa.tile([128, TB], DT, tag="t", name="psT")
                            nc.tensor.matmul(out=psT[:, :],
                                             lhsT=st["alpha"][:, b, :],
                                             rhs=identb[0:TB, 0:TB],
                                             start=True, stop=True)
                            nc.vector.tensor_copy(out=st["aT"][:, b, :],
                                                  in_=psT[:, :])
                        return f

                    def p_ctx(b, k):
                        def f():
                            if b == 0 and k == 0:
                                st["ctx"] = awp.tile([128, KC, TB, BL], BF,
                                                     tag="ctx", name="ctx")
                            psC = pa.tile([128, TB], DT, tag="c", name="psC")
                            nc.tensor.matmul(
                                out=psC[:, :], lhsT=ht_enc[:, b, k, :],
                                rhs=st["aT"][:, b, :], start=True, stop=True)
                            nc.vector.tensor_copy(
                                out=st["ctx"][:, k, :, b], in_=psC[:, :])
                        return f

                    def p_out(m):
                        def f():
                            psO = pa.tile([128, TB * BL], DT, tag="s",
                                          name="psO")
                            for k in range(KC):
                                nc.tensor.matmul(
                                    out=psO[:, :],
                                    lhsT=cwt_sb[:, k, m * 128:(m + 1) * 128],
                                    rhs=st["ctx"][:, k, :, :],
                                    start=(k == 0), stop=False)
                            nc.tensor.matmul(
                                out=psO[:, :], lhsT=identb[:, :],
                                rhs=hd_dec[:, m, t0 * BL:t0 * BL + TB * BL],
                                start=False, stop=True)
                            nc.vector.tensor_copy(
                                out=houts[:, m, j * 128:(j + 1) * 128],
                                in_=psO[:, :])
                            if m == KC - 1:
                                pending.extend((j, n0, nv)
                                               for (n0, nv) in dchunks)
                        return f

                    out = [p_scores(b) for b in range(BL)]
                    out.append(p_exp)
                    # per-b softmax->transpose->ctx so outproj unblocks early
                    for b in range(BL):
                        out.append(p_soft(b))
                        out.append(p_aT(b))
                        out.extend(p_ctx(b, k) for k in range(KC))
                    out.extend(p_out(m) for m in range(KC))
                    return out

                # projection: 1024-col double-chunks. The PSUM tile spans two
                # banks; each matmul stays within one bank; one wide copy
                # (alternating DVE/ACT) + one 1024-wide DMA per double-chunk.
                def emit_proj_chunk(j, n0, nv, engine):
                    plt = pl.tile([128, 1024], DT, tag="pl", name="plt")
                    for h0 in range(0, nv, 512):
                        hv = min(512, nv - h0)
                        for k in range(KC):
                            nc.tensor.matmul(
                                out=plt[:, h0:h0 + hv],
                                lhsT=houts[:, k, j * 128:(j + 1) * 128],
                                rhs=w_sb[:, k, n0 + h0:n0 + h0 + hv],
                                start=(k == 0), stop=(k == KC - 1))
                    lt = ltp.tile([128, 1024], BF, tag="lt", name="lt")
                    if engine == "act":
                        nc.scalar.copy(out=lt[:, :nv], in_=plt[:, :nv])
                    else:
                        nc.vector.tensor_copy(out=lt[:, :nv], in_=plt[:, :nv])
                    nc.sync.dma_start(
                        out=out_d[j * 128:(j + 1) * 128, n0:n0 + nv],
                        in_=lt[:, :nv])

                for t in range(T):
                    bkd = pd.tile([128, KC, BL], DT, tag="d")
                    for m in range(KC):
                        for k in range(KC):
                            rhs = (hT_sb[:, k, :] if t == 0
                                   else hd_dec[:, k, (t - 1) * BL:t * BL])
                            nc.tensor.matmul(
                                out=bkd[:, m, :],
                                lhsT=u_sb[:, k, m * 128:(m + 1) * 128],
                                rhs=rhs, start=(m == 0 and k == 0), stop=False)
                    nc.tensor.matmul(
                        out=bkd[:, :, :], lhsT=identb[:, :],
                        rhs=xt[:, t, :, :], start=False, stop=True)
                    nc.scalar.activation(
                        out=hd_dec[:, :, t * BL:(t + 1) * BL], in_=bkd[:, :, :],
                        func=AF.Tanh)
                    nc.gpsimd.tensor_copy(out=hd_dbt[:, :, :, t],
                                          in_=hd_dec[:, :, t * BL:(t + 1) * BL])
                    # H' -> H^T transposes ride the idle first decoder steps
                    # (attention only needs ht_enc from step TB onwards)
                    if t < BL * KC:
                        b, k = divmod(t, KC)
                        tp2 = pa.tile([128, 128], DT, tag="s", name="tp2")
                        nc.tensor.matmul(
                            out=tp2[:, :], lhsT=hd_enc[:, k, b, :],
                            rhs=identb[:, :], start=True, stop=True)
                        nc.vector.tensor_copy(
                            out=ht_enc[:, b, k, :], in_=tp2[:, :])
                    if t % TB == TB - 1:
                        pieces.extend(make_attention_pieces(t // TB))
                    if interleave:
                        npop = 2 if len(pending) > 40 else 1
                        for i in range(npop):
                            if pending:
                                eng = ("act" if (t % 2 == 0 and i == 0)
                                       else "dve")
                                emit_proj_chunk(*pending.popleft(), engine=eng)
                    if pieces:
                        pieces.popleft()()
                # drain: finish last block's attention pieces + projection
                i = 0
                while pieces or pending:
                    if pending:
                        eng = "act" if i % 2 == 0 else "dve"
                        emit_proj_chunk(*pending.popleft(), engine=eng)
                        i += 1
                    if pieces:
                        pieces.popleft()()
    nc.compile()
    return nc


def _prep_in_maps(U, b_enc1, b_enc2, b_dec, E_en, E_de, ctx_W, W_out_de,
                  src_en, tgt_de_in):
    f32 = np.float32
    U = np.ascontiguousarray(U, f32).astype(NPBF)
    ctx_wt = np.ascontiguousarray(np.asarray(ctx_W, f32).T).astype(NPBF)
    w_out_t = np.ascontiguousarray(np.asarray(W_out_de, f32).T).astype(NPBF)
    E_en = (np.asarray(E_en, f32) + np.asarray(b_enc1, f32)[None, :]).astype(NPBF)
    E_de = (np.asarray(E_de, f32) + np.asarray(b_dec, f32)[None, :]).astype(NPBF)
    b2 = np.asarray(b_enc2, f32).reshape(KC, 128).T  # [128, KC]
    b2b = np.repeat(b2[:, :, None], BL, axis=2).reshape(128, KC * BL).astype(NPBF)
    src = np.asarray(src_en).astype(np.int32)
    tgt = np.asarray(tgt_de_in).astype(np.int32)
    in_maps = []
    for i in range(NCORES):
        b0 = i * BL
        in_maps.append({
            "u": U, "ctx_wt": ctx_wt, "w_out_t": w_out_t,
            "e_en": E_en, "e_de": E_de, "b2b": b2b,
            "src_idx": np.ascontiguousarray(src[:, b0:b0 + BL]),
            "tgt_idx": np.ascontiguousarray(tgt[:, b0:b0 + BL]),
        })
    return in_maps


def kernel(U, b_enc1, b_enc2, b_dec, E_en, E_de, ctx_W, W_out_de,
           src_en, tgt_de_in, _trace=False, _raw=False, _ncores=NCORES):
    if "nc" not in _CACHE:
        _CACHE["nc"] = _build()
    nc = _CACHE["nc"]
    in_maps = _prep_in_maps(U, b_enc1, b_enc2, b_dec, E_en, E_de, ctx_W,
                            W_out_de, src_en, tgt_de_in)[:_ncores]
    res = run_bass_kernel_spmd(nc, in_maps, list(range(_ncores)), trace=_trace)
    if _raw:
        return res
    logits = np.empty((T, _ncores * BL, V), np.float32)
    for i in range(_ncores):
        logits[:, i * BL:(i + 1) * BL, :] = (
            res.results[i]["out"].astype(np.float32).reshape(T, BL, V))
    if _trace:
        return logits, res
    return logits


# revision 7
# speedup vs baseline: 1.0784x; 1.0784x over previous
"""Seq2seq RNN with attention on 8 TRN2 NeuronCores.

Data-parallel over batch (B=32 -> 4 per core). Structure:

- Attention is hoisted OUT of the decoder loop: the decoder recurrence
  h_t = tanh(h_{t-1}@U + x_t + b) does not depend on attention, so the h
  chain runs alone (5 tiny matmuls + 1 activation per step) and attention
  + context + output-projection are computed per 32-step block as batched
  matmuls, split into small pieces interleaved one-per-step with the
  decoder so the serial softmax chain never head-of-line-blocks the
  engine FIFOs.
- The 2-layer encoder runs layer 2 skewed one step behind layer 1, so
  each step issues ONE activation computing [h1(t), h2(t-1)] — the
  recurrence critical path is a single PE->ACT->PE round trip per step.
- All elementwise adds in the recurrences are folded into PSUM via
  identity matmuls (PE accumulates x_t / h1 / bias on top of U products);
  the DVE is off the critical path entirely.
- b_enc1 / b_dec are folded into the embedding tables on the host.
- W_out.T (d x V, bf16) is DMA-prefetched into SBUF during the encoder,
  chunked and gated behind the embedding gathers so the small setup
  transfers win the DMA queue.
- The final projection (the only big GEMM, 107us of PE time) is emitted
  interleaved with the decoder blocks; 1024-wide PSUM->SBUF logit copies
  alternate between DVE and ACT; output DMAs are 1024 columns wide.
"""

import numpy as np
from collections import deque

import concourse.bass as bass
import concourse.bacc as bacc
import concourse.tile as tile
from concourse import mybir
from concourse.bass_utils import run_bass_kernel_spmd
from concourse.masks import make_identity

D = 256
V = 32000
T = 128  # T_SRC == T_TGT == 128
B = 32
NCORES = 8
BL = B // NCORES  # 4 batch elements per core
KC = D // 128  # 2 d-chunks of 128
TB = 32  # decoder block size
NBLK = T // TB
DT = mybir.dt.float32
BF = mybir.dt.bfloat16
NPBF = mybir.dt.np(BF)
AF = mybir.ActivationFunctionType
ALU = mybir.AluOpType
AX = mybir.AxisListType

_CACHE = {}

# projection column chunks: pairs that share one 1024-wide output DMA
def _nchunks():
    out = []
    n0 = 0
    while n0 < V:
        out.append((n0, min(512, V - n0)))
        n0 += 512
    return out


def _build(interleave=True):
    nc = bacc.Bacc(None)

    u_d = nc.declare_dram_parameter("u", [D, D], BF, isOutput=False)
    cwt_d = nc.declare_dram_parameter("ctx_wt", [D, D], BF, isOutput=False)
    wot_d = nc.declare_dram_parameter("w_out_t", [D, V], BF, isOutput=False)
    een_d = nc.declare_dram_parameter("e_en", [V, D], BF, isOutput=False)
    ede_d = nc.declare_dram_parameter("e_de", [V, D], BF, isOutput=False)
    b2b_d = nc.declare_dram_parameter("b2b", [128, KC * BL], BF, isOutput=False)
    si_d = nc.declare_dram_parameter("src_idx", [T, BL], mybir.dt.int32, isOutput=False)
    ti_d = nc.declare_dram_parameter("tgt_idx", [T, BL], mybir.dt.int32, isOutput=False)
    out_d = nc.declare_dram_parameter("out", [T * BL, V], BF, isOutput=True)

    with tile.TileContext(nc) as tc:
        with (
            tc.tile_pool(name="persist", bufs=1) as pp,
            tc.tile_pool(name="work", bufs=4) as wp,
        ):
            # ---- persistent SBUF tiles ----
            u_sb = pp.tile([128, KC, D], BF, tag="u")
            cwt_sb = pp.tile([128, KC, D], BF, tag="cwt")
            w_sb = pp.tile([128, KC, V], BF, tag="w")  # W_out.T chunks
            ident = pp.tile([128, 128], DT, tag="ident")
            identb = pp.tile([128, 128], BF, tag="identb")
            ones1 = pp.tile([1, TB], BF, tag="ones1")
            b2b_sb = pp.tile([128, KC, BL], BF, tag="b2b")
            si_sb = pp.tile([T, BL], mybir.dt.int32, tag="si")
            ti_sb = pp.tile([T, BL], mybir.dt.int32, tag="ti")
            maddb = pp.tile([1, BL, T], BF, tag="maddb")  # -1e9 at PAD, on part 0
            xs = pp.tile([128, T, KC, BL], BF, tag="xs")  # x_src' [d, t, m, b]
            xt = pp.tile([128, T, KC, BL], BF, tag="xt")  # x_tgt'
            hd_enc = pp.tile([128, KC, BL, T], BF, tag="hd")  # H' [d,k,b,t]
            ht_enc = pp.tile([128, BL, KC, 128], BF, tag="ht")  # H^T [t,b,k,d]
            hcat = pp.tile([128, 2, KC, BL], BF, tag="hcat")  # enc state [layer,m,b]
            hT_sb = pp.tile([128, KC, BL], BF, tag="hT")  # enc final h2
            hd_dec = pp.tile([128, KC, T * BL], BF, tag="hdd")  # dec h', r=t*BL+b
            hd_dbt = pp.tile([128, KC, BL, T], BF, tag="hdbt")  # dec h' [d,k,b,t]
            houts = pp.tile([128, KC, T * BL], BF, tag="houts")  # outs'

            # ---- small constant loads (si/ti first: gathers depend on them;
            # u/cwt/b2b are only needed from encoder step 1, loaded later) ----
            nc.sync.dma_start(out=si_sb[:, :], in_=si_d[:, :])
            nc.sync.dma_start(out=ti_sb[:, :], in_=ti_d[:, :])
            make_identity(nc, ident[:, :])
            nc.vector.tensor_copy(out=identb[:, :], in_=ident[:, :])
            nc.vector.memset(ones1[:, :], 1.0)

            # ---- mask + embedding gathers + transposes ----
            with tc.tile_pool(name="pst", bufs=4, space="PSUM") as pst:
                mf = wp.tile([T, BL], DT, tag="mf")
                nc.vector.tensor_copy(out=mf[:, :], in_=si_sb[:, :])
                m01 = wp.tile([T, BL], DT, tag="m01")
                nc.vector.tensor_scalar(
                    out=m01[:, :], in0=mf[:, :], scalar1=0.0, scalar2=None,
                    op0=ALU.is_equal,
                )
                for b in range(BL):
                    psM = pst.tile([1, T], DT, tag="psM")
                    nc.tensor.matmul(out=psM[:, :], lhsT=m01[:, b:b + 1],
                                     rhs=ident[:, :], start=True, stop=True)
                    nc.vector.tensor_scalar(
                        out=maddb[:, b, :], in0=psM[:, :], scalar1=-1e9,
                        scalar2=None, op0=ALU.mult,
                    )

                for (idx_sb, e_d, xdst) in ((si_sb, een_d, xs), (ti_sb, ede_d, xt)):
                    for b in range(BL):
                        xg = wp.tile([T, D], BF, tag="xg", bufs=8)
                        nc.gpsimd.indirect_dma_start(
                            out=xg[:, :],
                            out_offset=None,
                            in_=e_d[:, :],
                            in_offset=bass.IndirectOffsetOnAxis(
                                ap=idx_sb[:, b:b + 1], axis=0),
                        )
                        for k in range(KC):
                            tp = pst.tile([128, 128], DT, tag="tp")
                            nc.tensor.matmul(
                                out=tp[:, :], lhsT=xg[:, k * 128:(k + 1) * 128],
                                rhs=identb[:, :], start=True, stop=True)
                            nc.vector.tensor_copy(out=xdst[:, :, k, b], in_=tp[:, :])

            for k in range(KC):
                nc.sync.dma_start(out=u_sb[:, k, :], in_=u_d[k * 128:(k + 1) * 128, :])
                nc.sync.dma_start(out=cwt_sb[:, k, :], in_=cwt_d[k * 128:(k + 1) * 128, :])
            nc.sync.dma_start(out=b2b_sb[:, :, :], in_=b2b_d[:, :])

            # ---- big weight prefetch. Tiny gpsimd writes into each chunk's
            # corner (reading the last gather output) force every weight DMA
            # to dispatch after the gathers, so the small transfers win the
            # DMA queue and the encoder starts immediately. ----
            WCH = 4000
            for w0 in range(0, V, WCH):
                for k in range(KC):
                    nc.gpsimd.tensor_copy(out=w_sb[0:1, k, w0:w0 + 4],
                                          in_=xt[0:1, T - 1, 0, 0:4])
            for w0 in range(0, V, WCH):
                for k in range(KC):
                    nc.sync.dma_start(
                        out=w_sb[:, k, w0:w0 + WCH],
                        in_=wot_d[k * 128:(k + 1) * 128, w0:w0 + WCH])

            # ---- encoder: layer 2 runs one step skewed so each step is ONE
            # activation computing [h1(t), h2(t-1)] — halves the ACT SEQ
            # serialization on the recurrence critical path. ----
            nc.vector.memset(hcat[:, 1, :, :], 0.0)  # h2(-1) = 0
            with tc.tile_pool(name="pe1", bufs=2, space="PSUM") as pe1:
                for t in range(T + 1):
                    if t == 0:
                        # h1(0) = tanh(x0) (b1 folded into E)
                        nc.scalar.activation(
                            out=hcat[:, 0, :, :], in_=xs[:, 0, :, :], func=AF.Tanh)
                        continue
                    bk1 = pe1.tile([128, 2, KC, BL], DT, tag="b1")
                    if t < T:
                        for m in range(KC):
                            for k in range(KC):
                                nc.tensor.matmul(
                                    out=bk1[:, 0, m, :],
                                    lhsT=u_sb[:, k, m * 128:(m + 1) * 128],
                                    rhs=hcat[:, 0, k, :],
                                    start=(m == 0 and k == 0), stop=False)
                        nc.tensor.matmul(
                            out=bk1[:, 0, :, :], lhsT=identb[:, :],
                            rhs=xs[:, t, :, :], start=False, stop=False)
                    for m in range(KC):
                        for k in range(KC):
                            nc.tensor.matmul(
                                out=bk1[:, 1, m, :],
                                lhsT=u_sb[:, k, m * 128:(m + 1) * 128],
                                rhs=hcat[:, 1, k, :],
                                start=(t == T and m == 0 and k == 0),
                                stop=False)
                    nc.tensor.matmul(
                        out=bk1[:, 1, :, :], lhsT=identb[:, :],
                        rhs=b2b_sb[:, :, :], start=False, stop=False)
                    nc.tensor.matmul(
                        out=bk1[:, 1, :, :], lhsT=identb[:, :],
                        rhs=hcat[:, 0, :, :], start=False, stop=True)
                    if t < T:
                        nc.scalar.activation(
                            out=hcat[:, :, :, :], in_=bk1[:, :, :, :],
                            func=AF.Tanh)
                    else:
                        nc.scalar.activation(
                            out=hcat[:, 1, :, :], in_=bk1[:, 1, :, :],
                            func=AF.Tanh)
                    nc.gpsimd.tensor_copy(out=hd_enc[:, :, :, t - 1],
                                          in_=hcat[:, 1, :, :])

            nc.gpsimd.tensor_copy(out=hT_sb[:, :, :], in_=hd_enc[:, :, :, T - 1])

            # ---- decoder + attention + projection (block-pipelined) ----
            dchunks = []
            n0 = 0
            while n0 < V:
                dchunks.append((n0, min(1024, V - n0)))
                n0 += 1024
            with (
                tc.tile_pool(name="pd", bufs=1, space="PSUM") as pd,
                tc.tile_pool(name="pa", bufs=1, space="PSUM") as pa,
                tc.tile_pool(name="pl", bufs=4, space="PSUM") as pl,
                tc.tile_pool(name="lt", bufs=4) as ltp,
                tc.tile_pool(name="aw", bufs=2) as awp,
            ):
                pending = deque()
                pieces = deque()

                def make_attention_pieces(j):
                    """Attention for block j as small thunks, interleaved one
                    per decoder step of block j+1 so the serial softmax chain
                    never head-of-line-blocks projection work in the FIFOs."""
                    t0 = j * TB
                    st = {}

                    def p_scores(b):
                        def f():
                            if b == 0:
                                st["psS"] = pa.tile([TB, BL, 128], DT, tag="s",
                                                    name="psS")
                            psS = st["psS"]
                            for k in range(KC):
                                nc.tensor.matmul(
                                    out=psS[:, b, :],
                                    lhsT=hd_dbt[:, k, b, t0:t0 + TB],
                                    rhs=hd_enc[:, k, b, :],
                                    start=(b == 0 and k == 0), stop=False)
                            nc.tensor.matmul(
                                out=psS[:, b, :], lhsT=ones1[:, :],
                                rhs=maddb[:, b, :],
                                start=False, stop=(b == BL - 1))
                        return f

                    def p_exp():
                        ex = awp.tile([TB, BL, 128], DT, tag="ex")
                        st["ex"] = ex
                        nc.scalar.activation(out=ex[:, :, :],
                                             in_=st["psS"][:, :, :],
                                             func=AF.Exp, scale=1.0 / 16.0)
                        st["alpha"] = awp.tile([TB, BL, 128], BF, tag="alpha",
                                               name="alpha")

                    def p_soft(b):
                        def f():
                            sm = wp.tile([TB, 1], DT, tag="sm")
                            nc.vector.reduce_sum(out=sm[:, :],
                                                 in_=st["ex"][:, b, :],
                                                 axis=AX.X)
                            rs = wp.tile([TB, 1], DT, tag="rs")
                            nc.vector.reciprocal(out=rs[:, :], in_=sm[:, :])
                            nc.vector.tensor_scalar(
                                out=st["alpha"][:, b, :], in0=st["ex"][:, b, :],
                                scalar1=rs[:, :1], scalar2=None, op0=ALU.mult)
                        return f

                    def p_aT(b):
                        def f():
                            if b == 0:
                                st["aT"] = awp.tile([128, BL, TB], BF,
                                                    tag="aT", name="aT")
                            psT = pa.tile([128, TB], DT, tag="t", name="psT")
                            nc.tensor.matmul(out=psT[:, :],
                                             lhsT=st["alpha"][:, b, :],
                                             rhs=identb[0:TB, 0:TB],
                                             start=True, stop=True)
                            nc.vector.tensor_copy(out=st["aT"][:, b, :],
                                                  in_=psT[:, :])
                        return f

                    def p_ctx(b, k):
                        def f():
                            if b == 0 and k == 0:
                                st["ctx"] = awp.tile([128, KC, TB, BL], BF,
                                                     tag="ctx", name="ctx")
                            psC = pa.tile([128, TB], DT, tag="c", name="psC")
                            nc.tensor.matmul(
                                out=psC[:, :], lhsT=ht_enc[:, b, k, :],
                                rhs=st["aT"][:, b, :], start=True, stop=True)
                            nc.vector.tensor_copy(
                                out=st["ctx"][:, k, :, b], in_=psC[:, :])
                        return f

                    def p_out(m):
                        def f():
                            psO = pa.tile([128, TB * BL], DT, tag="s",
                                          name="psO")
                            for k in range(KC):
                                nc.tensor.matmul(
                                    out=psO[:, :],
                                    lhsT=cwt_sb[:, k, m * 128:(m + 1) * 128],
                                    rhs=st["ctx"][:, k, :, :],
                                    start=(k == 0), stop=False)
                            nc.tensor.matmul(
                                out=psO[:, :], lhsT=identb[:, :],
                                rhs=hd_dec[:, m, t0 * BL:t0 * BL + TB * BL],
                                start=False, stop=True)
                            nc.vector.tensor_copy(
                                out=houts[:, m, j * 128:(j + 1) * 128],
                                in_=psO[:, :])
                            if m == KC - 1:
                                pending.extend((j, n0, nv)
                                               for (n0, nv) in dchunks)
                        return f

                    out = [p_scores(b) for b in range(BL)]
                    out.append(p_exp)
                    # per-b softmax->transpose->ctx so outproj unblocks early
                    for b in range(BL):
                        out.append(p_soft(b))
                        out.append(p_aT(b))
                        out.extend(p_ctx(b, k) for k in range(KC))
                    out.extend(p_out(m) for m in range(KC))
                    return out

                # projection: 1024-col double-chunks over four rotating
                # single-bank PSUM tiles; the two half-copies go to DVE and
                # ACT in parallel; one 1024-wide DMA per double-chunk.
                def emit_proj_chunk(j, n0, nv, engine):
                    lt = ltp.tile([128, 1024], BF, tag="lt", name="lt")
                    for h0 in range(0, nv, 512):
                        hv = min(512, nv - h0)
                        plt = pl.tile([128, 512], DT, tag="pl", name="plt")
                        for k in range(KC):
                            nc.tensor.matmul(
                                out=plt[:, :hv],
                                lhsT=houts[:, k, j * 128:(j + 1) * 128],
                                rhs=w_sb[:, k, n0 + h0:n0 + h0 + hv],
                                start=(k == 0), stop=(k == KC - 1))
                        if (h0 == 0) == (engine == "act"):
                            nc.scalar.copy(out=lt[:, h0:h0 + hv],
                                           in_=plt[:, :hv])
                        else:
                            nc.vector.tensor_copy(out=lt[:, h0:h0 + hv],
                                                  in_=plt[:, :hv])
                    nc.sync.dma_start(
                        out=out_d[j * 128:(j + 1) * 128, n0:n0 + nv],
                        in_=lt[:, :nv])

                for t in range(T):
                    bkd = pd.tile([128, KC, BL], DT, tag="d")
                    for m in range(KC):
                        for k in range(KC):
                            rhs = (hT_sb[:, k, :] if t == 0
                                   else hd_dec[:, k, (t - 1) * BL:t * BL])
                            nc.tensor.matmul(
                                out=bkd[:, m, :],
                                lhsT=u_sb[:, k, m * 128:(m + 1) * 128],
                                rhs=rhs, start=(m == 0 and k == 0), stop=False)
                    nc.tensor.matmul(
                        out=bkd[:, :, :], lhsT=identb[:, :],
                        rhs=xt[:, t, :, :], start=False, stop=True)
                    nc.scalar.activation(
                        out=hd_dec[:, :, t * BL:(t + 1) * BL], in_=bkd[:, :, :],
                        func=AF.Tanh)
                    nc.gpsimd.tensor_copy(out=hd_dbt[:, :, :, t],
                                          in_=hd_dec[:, :, t * BL:(t + 1) * BL])
                    # H' -> H^T transposes ride the idle first decoder steps
                    # (attention only needs ht_enc from step TB onwards)
                    if t < BL * KC:
                        b, k = divmod(t, KC)
                        tp2 = pa.tile([128, 128], DT, tag="s", name="tp2")
                        nc.tensor.matmul(
                            out=tp2[:, :], lhsT=hd_enc[:, k, b, :],
                            rhs=identb[:, :], start=True, stop=True)
                        nc.vector.tensor_copy(
                            out=ht_enc[:, b, k, :], in_=tp2[:, :])
                    if t % TB == TB - 1:
                        pieces.extend(make_attention_pieces(t // TB))
                    if interleave:
                        npop = 2 if len(pending) > 40 else 1
                        for i in range(npop):
                            if pending:
                                eng = ("act" if (t % 2 == 0 and i == 0)
                                       else "dve")
                                emit_proj_chunk(*pending.popleft(), engine=eng)
                    if pieces:
                        pieces.popleft()()
                # drain: finish last block's attention pieces + projection
                i = 0
                while pieces or pending:
                    if pending:
                        eng = "act" if i % 2 == 0 else "dve"
                        emit_proj_chunk(*pending.popleft(), engine=eng)
                        i += 1
                    if pieces:
                        pieces.popleft()()
    nc.compile()
    return nc


def _prep_in_maps(U, b_enc1, b_enc2, b_dec, E_en, E_de, ctx_W, W_out_de,
                  src_en, tgt_de_in):
    f32 = np.float32
    U = np.ascontiguousarray(U, f32).astype(NPBF)
    ctx_wt = np.ascontiguousarray(np.asarray(ctx_W, f32).T).astype(NPBF)
    w_out_t = np.ascontiguousarray(np.asarray(W_out_de, f32).T).astype(NPBF)
    E_en = (np.asarray(E_en, f32) + np.asarray(b_enc1, f32)[None, :]).astype(NPBF)
    E_de = (np.asarray(E_de, f32) + np.asarray(b_dec, f32)[None, :]).astype(NPBF)
    b2 = np.asarray(b_enc2, f32).reshape(KC, 128).T  # [128, KC]
    b2b = np.repeat(b2[:, :, None], BL, axis=2).reshape(128, KC * BL).astype(NPBF)
    src = np.asarray(src_en).astype(np.int32)
    tgt = np.asarray(tgt_de_in).astype(np.int32)
    in_maps = []
    for i in range(NCORES):
        b0 = i * BL
        in_maps.append({
            "u": U, "ctx_wt": ctx_wt, "w_out_t": w_out_t,
            "e_en": E_en, "e_de": E_de, "b2b": b2b,
            "src_idx": np.ascontiguousarray(src[:, b0:b0 + BL]),
            "tgt_idx": np.ascontiguousarray(tgt[:, b0:b0 + BL]),
        })
    return in_maps


def kernel(U, b_enc1, b_enc2, b_dec, E_en, E_de, ctx_W, W_out_de,
           src_en, tgt_de_in, _trace=False, _raw=False, _ncores=NCORES):
    if "nc" not in _CACHE:
        _CACHE["nc"] = _build()
    nc = _CACHE["nc"]
    in_maps = _prep_in_maps(U, b_enc1, b_enc2, b_dec, E_en, E_de, ctx_W,
                            W_out_de, src_en, tgt_de_in)[:_ncores]
    res = run_bass_kernel_spmd(nc, in_maps, list(range(_ncores)), trace=_trace)
    if _raw:
        return res
    logits = np.empty((T, _ncores * BL, V), np.float32)
    for i in range(_ncores):
        logits[:, i * BL:(i + 1) * BL, :] = (
            res.results[i]["out"].astype(np.float32).reshape(T, BL, V))
    if _trace:
        return logits, res
    return logits


# revision 9
# speedup vs baseline: 1.0828x; 1.0040x over previous
"""Seq2seq RNN with attention on 8 TRN2 NeuronCores.

Data-parallel over batch (B=32 -> 4 per core). Structure:

- Attention is hoisted OUT of the decoder loop: the decoder recurrence
  h_t = tanh(h_{t-1}@U + x_t + b) does not depend on attention, so the h
  chain runs alone (5 tiny matmuls + 1 activation per step) and attention
  + context + output-projection are computed per 32-step block as batched
  matmuls, split into small pieces interleaved one-per-step with the
  decoder so the serial softmax chain never head-of-line-blocks the
  engine FIFOs.
- The 2-layer encoder runs layer 2 skewed one step behind layer 1, so
  each step issues ONE activation computing [h1(t), h2(t-1)] — the
  recurrence critical path is a single PE->ACT->PE round trip per step.
- All elementwise adds in the recurrences are folded into PSUM via
  identity matmuls (PE accumulates x_t / h1 / bias on top of U products);
  the DVE is off the critical path entirely.
- b_enc1 / b_dec are folded into the embedding tables on the host.
- W_out.T (d x V, bf16) is DMA-prefetched into SBUF during the encoder,
  chunked and gated behind the embedding gathers so the small setup
  transfers win the DMA queue.
- The final projection (the only big GEMM, 107us of PE time) is emitted
  interleaved with the decoder blocks; 1024-wide PSUM->SBUF logit copies
  alternate between DVE and ACT; output DMAs are 1024 columns wide.
"""

import numpy as np
from collections import deque

import concourse.bass as bass
import concourse.bacc as bacc
import concourse.tile as tile
from concourse import mybir
from concourse.bass_utils import run_bass_kernel_spmd
from concourse.masks import make_identity

D = 256
V = 32000
T = 128  # T_SRC == T_TGT == 128
B = 32
NCORES = 8
BL = B // NCORES  # 4 batch elements per core
KC = D // 128  # 2 d-chunks of 128
TB = 32  # decoder block size
NBLK = T // TB
DT = mybir.dt.float32
BF = mybir.dt.bfloat16
NPBF = mybir.dt.np(BF)
AF = mybir.ActivationFunctionType
ALU = mybir.AluOpType
AX = mybir.AxisListType

_CACHE = {}

# projection column chunks: pairs that share one 1024-wide output DMA
def _nchunks():
    out = []
    n0 = 0
    while n0 < V:
        out.append((n0, min(512, V - n0)))
        n0 += 512
    return out


def _build(interleave=True):
    nc = bacc.Bacc(None)

    u_d = nc.declare_dram_parameter("u", [D, D], BF, isOutput=False)
    cwt_d = nc.declare_dram_parameter("ctx_wt", [D, D], BF, isOutput=False)
    wot_d = nc.declare_dram_parameter("w_out_t", [D, V], BF, isOutput=False)
    een_d = nc.declare_dram_parameter("e_en", [V, D], BF, isOutput=False)
    ede_d = nc.declare_dram_parameter("e_de", [V, D], BF, isOutput=False)
    b2b_d = nc.declare_dram_parameter("b2b", [128, KC * BL], BF, isOutput=False)
    si_d = nc.declare_dram_parameter("src_idx", [T, BL], mybir.dt.int32, isOutput=False)
    ti_d = nc.declare_dram_parameter("tgt_idx", [T, BL], mybir.dt.int32, isOutput=False)
    out_d = nc.declare_dram_parameter("out", [T * BL, V], BF, isOutput=True)

    with tile.TileContext(nc) as tc:
        with (
            tc.tile_pool(name="persist", bufs=1) as pp,
            tc.tile_pool(name="work", bufs=4) as wp,
        ):
            # ---- persistent SBUF tiles ----
            u_sb = pp.tile([128, KC, D], BF, tag="u")
            cwt_sb = pp.tile([128, KC, D], BF, tag="cwt")
            w_sb = pp.tile([128, KC, V], BF, tag="w")  # W_out.T chunks
            ident = pp.tile([128, 128], DT, tag="ident")
            identb = pp.tile([128, 128], BF, tag="identb")
            ones1 = pp.tile([1, TB], BF, tag="ones1")
            b2b_sb = pp.tile([128, KC, BL], BF, tag="b2b")
            si_sb = pp.tile([T, BL], mybir.dt.int32, tag="si")
            ti_sb = pp.tile([T, BL], mybir.dt.int32, tag="ti")
            maddb = pp.tile([1, BL, T], BF, tag="maddb")  # -1e9 at PAD, on part 0
            xs = pp.tile([128, T, KC, BL], BF, tag="xs")  # x_src' [d, t, m, b]
            xt = pp.tile([128, T, KC, BL], BF, tag="xt")  # x_tgt'
            hd_enc = pp.tile([128, KC, BL, T], BF, tag="hd")  # H' [d,k,b,t]
            ht_enc = pp.tile([128, BL, KC, 128], BF, tag="ht")  # H^T [t,b,k,d]
            hcat = pp.tile([128, 2, KC, BL], BF, tag="hcat")  # enc state [layer,m,b]
            hT_sb = pp.tile([128, KC, BL], BF, tag="hT")  # enc final h2
            hd_dec = pp.tile([128, KC, T * BL], BF, tag="hdd")  # dec h', r=t*BL+b
            hd_dbt = pp.tile([128, KC, BL, T], BF, tag="hdbt")  # dec h' [d,k,b,t]
            houts = pp.tile([128, KC, T * BL], BF, tag="houts")  # outs'

            # ---- small constant loads (si/ti first: gathers depend on them;
            # u/cwt/b2b are only needed from encoder step 1, loaded later) ----
            nc.sync.dma_start(out=si_sb[:, :], in_=si_d[:, :])
            nc.sync.dma_start(out=ti_sb[:, :], in_=ti_d[:, :])
            make_identity(nc, ident[:, :])
            nc.vector.tensor_copy(out=identb[:, :], in_=ident[:, :])
            nc.vector.memset(ones1[:, :], 1.0)
            # dummy activation: pulls the ~2.7us ACT table load (tanh/exp
            # share one set) into the setup phase instead of encoder t=0
            warm = wp.tile([1, 1], DT, tag="warm")
            nc.scalar.activation(out=warm[:, :], in_=ident[0:1, 0:1],
                                 func=AF.Tanh)

            # ---- mask + embedding gathers + transposes ----
            with tc.tile_pool(name="pst", bufs=4, space="PSUM") as pst:
                mf = wp.tile([T, BL], DT, tag="mf")
                nc.vector.tensor_copy(out=mf[:, :], in_=si_sb[:, :])
                m01 = wp.tile([T, BL], DT, tag="m01")
                nc.vector.tensor_scalar(
                    out=m01[:, :], in0=mf[:, :], scalar1=0.0, scalar2=None,
                    op0=ALU.is_equal,
                )
                for b in range(BL):
                    psM = pst.tile([1, T], DT, tag="psM")
                    nc.tensor.matmul(out=psM[:, :], lhsT=m01[:, b:b + 1],
                                     rhs=ident[:, :], start=True, stop=True)
                    nc.vector.tensor_scalar(
                        out=maddb[:, b, :], in0=psM[:, :], scalar1=-1e9,
                        scalar2=None, op0=ALU.mult,
                    )

                for (idx_sb, e_d, xdst) in ((si_sb, een_d, xs), (ti_sb, ede_d, xt)):
                    for b in range(BL):
                        xg = wp.tile([T, D], BF, tag="xg", bufs=8)
                        nc.gpsimd.indirect_dma_start(
                            out=xg[:, :],
                            out_offset=None,
                            in_=e_d[:, :],
                            in_offset=bass.IndirectOffsetOnAxis(
                                ap=idx_sb[:, b:b + 1], axis=0),
                        )
                        for k in range(KC):
                            tp = pst.tile([128, 128], DT, tag="tp")
                            nc.tensor.matmul(
                                out=tp[:, :], lhsT=xg[:, k * 128:(k + 1) * 128],
                                rhs=identb[:, :], start=True, stop=True)
                            nc.vector.tensor_copy(out=xdst[:, :, k, b], in_=tp[:, :])

            for k in range(KC):
                nc.sync.dma_start(out=u_sb[:, k, :], in_=u_d[k * 128:(k + 1) * 128, :])
                nc.sync.dma_start(out=cwt_sb[:, k, :], in_=cwt_d[k * 128:(k + 1) * 128, :])
            nc.sync.dma_start(out=b2b_sb[:, :, :], in_=b2b_d[:, :])

            # ---- big weight prefetch. Tiny gpsimd writes into each chunk's
            # corner (reading the last gather output) force every weight DMA
            # to dispatch after the gathers, so the small transfers win the
            # DMA queue and the encoder starts immediately. ----
            WCH = 4000
            for w0 in range(0, V, WCH):
                for k in range(KC):
                    nc.gpsimd.tensor_copy(out=w_sb[0:1, k, w0:w0 + 4],
                                          in_=xt[0:1, T - 1, 0, 0:4])
            for w0 in range(0, V, WCH):
                for k in range(KC):
                    nc.sync.dma_start(
                        out=w_sb[:, k, w0:w0 + WCH],
                        in_=wot_d[k * 128:(k + 1) * 128, w0:w0 + WCH])

            # ---- encoder: layer 2 runs one step skewed so each step is ONE
            # activation computing [h1(t), h2(t-1)] — halves the ACT SEQ
            # serialization on the recurrence critical path. ----
            nc.vector.memset(hcat[:, 1, :, :], 0.0)  # h2(-1) = 0
            with tc.tile_pool(name="pe1", bufs=2, space="PSUM") as pe1:
                for t in range(T + 1):
                    if t == 0:
                        # h1(0) = tanh(x0) (b1 folded into E)
                        nc.scalar.activation(
                            out=hcat[:, 0, :, :], in_=xs[:, 0, :, :], func=AF.Tanh)
                        continue
                    bk1 = pe1.tile([128, 2, KC, BL], DT, tag="b1")
                    if t < T:
                        for m in range(KC):
                            for k in range(KC):
                                nc.tensor.matmul(
                                    out=bk1[:, 0, m, :],
                                    lhsT=u_sb[:, k, m * 128:(m + 1) * 128],
                                    rhs=hcat[:, 0, k, :],
                                    start=(m == 0 and k == 0), stop=False)
                        nc.tensor.matmul(
                            out=bk1[:, 0, :, :], lhsT=identb[:, :],
                            rhs=xs[:, t, :, :], start=False, stop=False)
                    for m in range(KC):
                        for k in range(KC):
                            nc.tensor.matmul(
                                out=bk1[:, 1, m, :],
                                lhsT=u_sb[:, k, m * 128:(m + 1) * 128],
                                rhs=hcat[:, 1, k, :],
                                start=(t == T and m == 0 and k == 0),
                                stop=False)
                    nc.tensor.matmul(
                        out=bk1[:, 1, :, :], lhsT=identb[:, :],
                        rhs=b2b_sb[:, :, :], start=False, stop=False)
                    nc.tensor.matmul(
                        out=bk1[:, 1, :, :], lhsT=identb[:, :],
                        rhs=hcat[:, 0, :, :], start=False, stop=True)
                    if t < T:
                        nc.scalar.activation(
                            out=hcat[:, :, :, :], in_=bk1[:, :, :, :],
                            func=AF.Tanh)
                    else:
                        nc.scalar.activation(
                            out=hcat[:, 1, :, :], in_=bk1[:, 1, :, :],
                            func=AF.Tanh)
                    nc.gpsimd.tensor_copy(out=hd_enc[:, :, :, t - 1],
                                          in_=hcat[:, 1, :, :])

            nc.gpsimd.tensor_copy(out=hT_sb[:, :, :], in_=hd_enc[:, :, :, T - 1])

            # ---- decoder + attention + projection (block-pipelined) ----
            dchunks = []
            n0 = 0
            while n0 < V:
                dchunks.append((n0, min(1024, V - n0)))
                n0 += 1024
            with (
                tc.tile_pool(name="pd", bufs=1, space="PSUM") as pd,
                tc.tile_pool(name="pa", bufs=1, space="PSUM") as pa,
                tc.tile_pool(name="pl", bufs=4, space="PSUM") as pl,
                tc.tile_pool(name="lt", bufs=4) as ltp,
                tc.tile_pool(name="aw", bufs=2) as awp,
            ):
                pending = deque()
                pieces = deque()

                def make_attention_pieces(j):
                    """Attention for block j as small thunks, interleaved one
                    per decoder step of block j+1 so the serial softmax chain
                    never head-of-line-blocks projection work in the FIFOs."""
                    t0 = j * TB
                    st = {}

                    def p_scores(b):
                        def f():
                            if b == 0:
                                st["psS"] = pa.tile([TB, BL, 128], DT, tag="s",
                                                    name="psS")
                            psS = st["psS"]
                            for k in range(KC):
                                nc.tensor.matmul(
                                    out=psS[:, b, :],
                                    lhsT=hd_dbt[:, k, b, t0:t0 + TB],
                                    rhs=hd_enc[:, k, b, :],
                                    start=(b == 0 and k == 0), stop=False)
                            nc.tensor.matmul(
                                out=psS[:, b, :], lhsT=ones1[:, :],
                                rhs=maddb[:, b, :],
                                start=False, stop=(b == BL - 1))
                        return f

                    def p_exp():
                        ex = awp.tile([TB, BL, 128], DT, tag="ex")
                        st["ex"] = ex
                        nc.scalar.activation(out=ex[:, :, :],
                                             in_=st["psS"][:, :, :],
                                             func=AF.Exp, scale=1.0 / 16.0)
                        st["alpha"] = awp.tile([TB, BL, 128], BF, tag="alpha",
                                               name="alpha")

                    def p_soft(b):
                        def f():
                            sm = wp.tile([TB, 1], DT, tag="sm")
                            nc.vector.reduce_sum(out=sm[:, :],
                                                 in_=st["ex"][:, b, :],
                                                 axis=AX.X)
                            rs = wp.tile([TB, 1], DT, tag="rs")
                            nc.vector.reciprocal(out=rs[:, :], in_=sm[:, :])
                            nc.vector.tensor_scalar(
                                out=st["alpha"][:, b, :], in0=st["ex"][:, b, :],
                                scalar1=rs[:, :1], scalar2=None, op0=ALU.mult)
                        return f

                    def p_aT(b):
                        def f():
                            if b == 0:
                                st["aT"] = awp.tile([128, BL, TB], BF,
                                                    tag="aT", name="aT")
                            psT = pa.tile([128, TB], DT, tag="t", name="psT")
                            nc.tensor.matmul(out=psT[:, :],
                                             lhsT=st["alpha"][:, b, :],
                                             rhs=identb[0:TB, 0:TB],
                                             start=True, stop=True)
                            nc.vector.tensor_copy(out=st["aT"][:, b, :],
                                                  in_=psT[:, :])
                        return f

                    def p_ctx(b, k):
                        def f():
                            if b == 0 and k == 0:
                                st["ctx"] = awp.tile([128, KC, TB, BL], BF,
                                                     tag="ctx", name="ctx")
                            psC = pa.tile([128, TB], DT, tag="c", name="psC")
                            nc.tensor.matmul(
                                out=psC[:, :], lhsT=ht_enc[:, b, k, :],
                                rhs=st["aT"][:, b, :], start=True, stop=True)
                            nc.vector.tensor_copy(
                                out=st["ctx"][:, k, :, b], in_=psC[:, :])
                        return f

                    def p_out(m):
                        def f():
                            psO = pa.tile([128, TB * BL], DT, tag="s",
                                          name="psO")
                            for k in range(KC):
                                nc.tensor.matmul(
                                    out=psO[:, :],
                                    lhsT=cwt_sb[:, k, m * 128:(m + 1) * 128],
                                    rhs=st["ctx"][:, k, :, :],
                                    start=(k == 0), stop=False)
                            nc.tensor.matmul(
                                out=psO[:, :], lhsT=identb[:, :],
                                rhs=hd_dec[:, m, t0 * BL:t0 * BL + TB * BL],
                                start=False, stop=True)
                            nc.vector.tensor_copy(
                                out=houts[:, m, j * 128:(j + 1) * 128],
                                in_=psO[:, :])
                            if m == KC - 1:
                                pending.extend((j, n0, nv)
                                               for (n0, nv) in dchunks)
                        return f

                    out = [p_scores(b) for b in range(BL)]
                    out.append(p_exp)
                    # per-b softmax->transpose->ctx so outproj unblocks early
                    for b in range(BL):
                        out.append(p_soft(b))
                        out.append(p_aT(b))
                        out.extend(p_ctx(b, k) for k in range(KC))
                    out.extend(p_out(m) for m in range(KC))
                    return out

                # projection: 1024-col double-chunks over four rotating
                # single-bank PSUM tiles; the two half-copies go to DVE and
                # ACT in parallel; one 1024-wide DMA per double-chunk.
                def emit_proj_chunk(j, n0, nv, engine):
                    lt = ltp.tile([128, 1024], BF, tag="lt", name="lt")
                    for h0 in range(0, nv, 512):
                        hv = min(512, nv - h0)
                        plt = pl.tile([128, 512], DT, tag="pl", name="plt")
                        for k in range(KC):
                            nc.tensor.matmul(
                                out=plt[:, :hv],
                                lhsT=houts[:, k, j * 128:(j + 1) * 128],
                                rhs=w_sb[:, k, n0 + h0:n0 + h0 + hv],
                                start=(k == 0), stop=(k == KC - 1))
                        if (h0 == 0) == (engine == "act"):
                            nc.scalar.copy(out=lt[:, h0:h0 + hv],
                                           in_=plt[:, :hv])
                        else:
                            nc.vector.tensor_copy(out=lt[:, h0:h0 + hv],
                                                  in_=plt[:, :hv])
                    nc.sync.dma_start(
                        out=out_d[j * 128:(j + 1) * 128, n0:n0 + nv],
                        in_=lt[:, :nv])

                for t in range(T):
                    bkd = pd.tile([128, KC, BL], DT, tag="d")
                    for m in range(KC):
                        for k in range(KC):
                            rhs = (hT_sb[:, k, :] if t == 0
                                   else hd_dec[:, k, (t - 1) * BL:t * BL])
                            nc.tensor.matmul(
                                out=bkd[:, m, :],
                                lhsT=u_sb[:, k, m * 128:(m + 1) * 128],
                                rhs=rhs, start=(m == 0 and k == 0), stop=False)
                    nc.tensor.matmul(
                        out=bkd[:, :, :], lhsT=identb[:, :],
                        rhs=xt[:, t, :, :], start=False, stop=True)
                    nc.scalar.activation(
                        out=hd_dec[:, :, t * BL:(t + 1) * BL], in_=bkd[:, :, :],
                        func=AF.Tanh)
                    nc.gpsimd.tensor_copy(out=hd_dbt[:, :, :, t],
                                          in_=hd_dec[:, :, t * BL:(t + 1) * BL])
                    # H' -> H^T transposes ride the idle first decoder steps
                    # (attention only needs ht_enc from step TB onwards)
                    if t < BL * KC:
                        b, k = divmod(t, KC)
                        tp2 = pa.tile([128, 128], DT, tag="s", name="tp2")
                        nc.tensor.matmul(
                            out=tp2[:, :], lhsT=hd_enc[:, k, b, :],
                            rhs=identb[:, :], start=True, stop=True)
                        nc.vector.tensor_copy(
                            out=ht_enc[:, b, k, :], in_=tp2[:, :])
                    if t % TB == TB - 1:
                        pieces.extend(make_attention_pieces(t // TB))
                        # kick the scores pieces immediately (they only
                        # need this block's now-complete hidden states)
                        for _ in range(4):
                            pieces.popleft()()
                    if interleave:
                        npop = 2 if len(pending) > 40 else 1
                        for i in range(npop):
                            if pending:
                                eng = ("act" if (t % 2 == 0 and i == 0)
                                       else "dve")
                                emit_proj_chunk(*pending.popleft(), engine=eng)
                    if pieces:
                        pieces.popleft()()
                # drain: finish last block's attention pieces + projection
                i = 0
                while pieces or pending:
                    if pending:
                        eng = "act" if i % 2 == 0 else "dve"
                        emit_proj_chunk(*pending.popleft(), engine=eng)
                        i += 1
                    if pieces:
                        pieces.popleft()()
    nc.compile()
    return nc


def _prep_in_maps(U, b_enc1, b_enc2, b_dec, E_en, E_de, ctx_W, W_out_de,
                  src_en, tgt_de_in):
    f32 = np.float32
    U = np.ascontiguousarray(U, f32).astype(NPBF)
    ctx_wt = np.ascontiguousarray(np.asarray(ctx_W, f32).T).astype(NPBF)
    w_out_t = np.ascontiguousarray(np.asarray(W_out_de, f32).T).astype(NPBF)
    E_en = (np.asarray(E_en, f32) + np.asarray(b_enc1, f32)[None, :]).astype(NPBF)
    E_de = (np.asarray(E_de, f32) + np.asarray(b_dec, f32)[None, :]).astype(NPBF)
    b2 = np.asarray(b_enc2, f32).reshape(KC, 128).T  # [128, KC]
    b2b = np.repeat(b2[:, :, None], BL, axis=2).reshape(128, KC * BL).astype(NPBF)
    src = np.asarray(src_en).astype(np.int32)
    tgt = np.asarray(tgt_de_in).astype(np.int32)
    in_maps = []
    for i in range(NCORES):
        b0 = i * BL
        in_maps.append({
            "u": U, "ctx_wt": ctx_wt, "w_out_t": w_out_t,
            "e_en": E_en, "e_de": E_de, "b2b": b2b,
            "src_idx": np.ascontiguousarray(src[:, b0:b0 + BL]),
            "tgt_idx": np.ascontiguousarray(tgt[:, b0:b0 + BL]),
        })
    return in_maps


def kernel(U, b_enc1, b_enc2, b_dec, E_en, E_de, ctx_W, W_out_de,
           src_en, tgt_de_in, _trace=False, _raw=False, _ncores=NCORES):
    if "nc" not in _CACHE:
        _CACHE["nc"] = _build()
    nc = _CACHE["nc"]
    in_maps = _prep_in_maps(U, b_enc1, b_enc2, b_dec, E_en, E_de, ctx_W,
                            W_out_de, src_en, tgt_de_in)[:_ncores]
    res = run_bass_kernel_spmd(nc, in_maps, list(range(_ncores)), trace=_trace)
    if _raw:
        return res
    logits = np.empty((T, _ncores * BL, V), np.float32)
    for i in range(_ncores):
        logits[:, i * BL:(i + 1) * BL, :] = (
            res.results[i]["out"].astype(np.float32).reshape(T, BL, V))
    if _trace:
        return logits, res
    return logits


# revision 12
# speedup vs baseline: 1.1253x; 1.0393x over previous
"""Seq2seq RNN with attention on 8 TRN2 NeuronCores.

Data-parallel over batch (B=32 -> 4 per core). Structure:

- Attention is hoisted OUT of the decoder loop (the recurrence doesn't
  depend on it) and computed per 32-step block as batched matmuls, split
  into small pieces interleaved one-per-step with the decoder so the
  serial softmax chain never head-of-line-blocks the engine FIFOs.
- The 2-layer encoder runs layer 2 skewed one step behind layer 1: each
  step is ONE activation computing [h1(t), h2(t-1)], written to fresh
  columns of a state-history tile so the chain has no WAR edges — a
  single PE->ACT->PE semaphore round trip (~700ns) per step.
- All elementwise adds are folded into PSUM via identity-matmul
  accumulation; b_enc1/b_dec are folded into the embeddings on the host.
- W_out.T (bf16) is DMA-prefetched into SBUF during the encoder, chunked
  and gated behind the embedding gathers.
- The final projection (107us of PE work) is interleaved with the
  decoder blocks; each 1024-col logit chunk's two halves are copied
  PSUM->SBUF on DVE and ACT in parallel over four rotating single-bank
  PSUM buffers; output DMAs are 1024 columns wide.
"""

import numpy as np
from collections import deque

import concourse.bass as bass
import concourse.bacc as bacc
import concourse.tile as tile
from concourse import mybir
from concourse.bass_utils import run_bass_kernel_spmd
from concourse.masks import make_identity

D = 256
V = 32000
T = 128  # T_SRC == T_TGT == 128
B = 32
NCORES = 8
BL = B // NCORES  # 4 batch elements per core
KC = D // 128  # 2 d-chunks of 128
TB = 32  # decoder block size
NBLK = T // TB
DT = mybir.dt.float32
BF = mybir.dt.bfloat16
NPBF = mybir.dt.np(BF)
AF = mybir.ActivationFunctionType
ALU = mybir.AluOpType
AX = mybir.AxisListType

_CACHE = {}

# projection column chunks: pairs that share one 1024-wide output DMA
def _nchunks():
    out = []
    n0 = 0
    while n0 < V:
        out.append((n0, min(512, V - n0)))
        n0 += 512
    return out


def _build(interleave=True):
    nc = bacc.Bacc(None)

    u_d = nc.declare_dram_parameter("u", [D, D], BF, isOutput=False)
    cwt_d = nc.declare_dram_parameter("ctx_wt", [D, D], BF, isOutput=False)
    wot_d = nc.declare_dram_parameter("w_out_t", [D, V], BF, isOutput=False)
    een_d = nc.declare_dram_parameter("e_en", [V, D], BF, isOutput=False)
    ede_d = nc.declare_dram_parameter("e_de", [V, D], BF, isOutput=False)
    b2b_d = nc.declare_dram_parameter("b2b", [128, KC * BL], BF, isOutput=False)
    si_d = nc.declare_dram_parameter("src_idx", [T, BL], mybir.dt.int32, isOutput=False)
    ti_d = nc.declare_dram_parameter("tgt_idx", [T, BL], mybir.dt.int32, isOutput=False)
    out_d = nc.declare_dram_parameter("out", [T * BL, V], BF, isOutput=True)

    with tile.TileContext(nc) as tc:
        with (
            tc.tile_pool(name="persist", bufs=1) as pp,
            tc.tile_pool(name="work", bufs=4) as wp,
        ):
            # ---- persistent SBUF tiles ----
            u_sb = pp.tile([128, KC, D], BF, tag="u")
            cwt_sb = pp.tile([128, KC, D], BF, tag="cwt")
            w_sb = pp.tile([128, KC, V], BF, tag="w")  # W_out.T chunks
            ident = pp.tile([128, 128], DT, tag="ident")
            identb = pp.tile([128, 128], BF, tag="identb")
            ones1 = pp.tile([1, TB], BF, tag="ones1")
            b2b_sb = pp.tile([128, KC, BL], BF, tag="b2b")
            si_sb = pp.tile([T, BL], mybir.dt.int32, tag="si")
            ti_sb = pp.tile([T, BL], mybir.dt.int32, tag="ti")
            maddb = pp.tile([1, BL, T], BF, tag="maddb")  # -1e9 at PAD, on part 0
            xs = pp.tile([128, T, KC, BL], BF, tag="xs")  # x_src' [d, t, m, b]
            xt = pp.tile([128, T, KC, BL], BF, tag="xt")  # x_tgt'
            hd_enc = pp.tile([128, KC, BL, T], BF, tag="hd")  # H' [d,k,b,t]
            ht_enc = pp.tile([128, BL, KC, 128], BF, tag="ht")  # H^T [t,b,k,d]
            # encoder state history: slot [t,0]=h1(t), [t,1]=h2(t-1) — every
            # activation writes FRESH columns, so there is no WAR edge on the
            # recurrence chain (matches the decoder's 652ns/step rhythm)
            enc_all = pp.tile([128, T + 1, 2, KC, BL], BF, tag="enc_all")
            hT_sb = pp.tile([128, KC, BL], BF, tag="hT")  # enc final h2
            hd_dec = pp.tile([128, KC, T * BL], BF, tag="hdd")  # dec h', r=t*BL+b
            hd_dbt = pp.tile([128, KC, BL, T], BF, tag="hdbt")  # dec h' [d,k,b,t]
            houts = pp.tile([128, KC, T * BL], BF, tag="houts")  # outs'

            # ---- small constant loads (si/ti first: gathers depend on them;
            # u/cwt/b2b are only needed from encoder step 1, loaded later) ----
            nc.sync.dma_start(out=si_sb[:, :], in_=si_d[:, :])
            nc.sync.dma_start(out=ti_sb[:, :], in_=ti_d[:, :])
            make_identity(nc, ident[:, :])
            nc.vector.tensor_copy(out=identb[:, :], in_=ident[:, :])
            nc.vector.memset(ones1[:, :], 1.0)
            # dummy activation: pulls the ~2.7us ACT table load (tanh/exp
            # share one set) into the setup phase instead of encoder t=0
            warm = wp.tile([1, 1], DT, tag="warm")
            nc.scalar.activation(out=warm[:, :], in_=ident[0:1, 0:1],
                                 func=AF.Tanh)

            # ---- mask + embedding gathers + transposes ----
            with tc.tile_pool(name="pst", bufs=4, space="PSUM") as pst:
                mf = wp.tile([T, BL], DT, tag="mf")
                nc.vector.tensor_copy(out=mf[:, :], in_=si_sb[:, :])
                m01 = wp.tile([T, BL], DT, tag="m01")
                nc.vector.tensor_scalar(
                    out=m01[:, :], in0=mf[:, :], scalar1=0.0, scalar2=None,
                    op0=ALU.is_equal,
                )
                for b in range(BL):
                    psM = pst.tile([1, T], DT, tag="psM")
                    nc.tensor.matmul(out=psM[:, :], lhsT=m01[:, b:b + 1],
                                     rhs=ident[:, :], start=True, stop=True)
                    nc.vector.tensor_scalar(
                        out=maddb[:, b, :], in0=psM[:, :], scalar1=-1e9,
                        scalar2=None, op0=ALU.mult,
                    )

                for (idx_sb, e_d, xdst) in ((si_sb, een_d, xs), (ti_sb, ede_d, xt)):
                    for b in range(BL):
                        xg = wp.tile([T, D], BF, tag="xg", bufs=8)
                        nc.gpsimd.indirect_dma_start(
                            out=xg[:, :],
                            out_offset=None,
                            in_=e_d[:, :],
                            in_offset=bass.IndirectOffsetOnAxis(
                                ap=idx_sb[:, b:b + 1], axis=0),
                        )
                        for k in range(KC):
                            tp = pst.tile([128, 128], DT, tag="tp")
                            nc.tensor.matmul(
                                out=tp[:, :], lhsT=xg[:, k * 128:(k + 1) * 128],
                                rhs=identb[:, :], start=True, stop=True)
                            nc.vector.tensor_copy(out=xdst[:, :, k, b], in_=tp[:, :])

            for k in range(KC):
                nc.sync.dma_start(out=u_sb[:, k, :], in_=u_d[k * 128:(k + 1) * 128, :])
                nc.sync.dma_start(out=cwt_sb[:, k, :], in_=cwt_d[k * 128:(k + 1) * 128, :])
            nc.sync.dma_start(out=b2b_sb[:, :, :], in_=b2b_d[:, :])

            # ---- big weight prefetch. Tiny gpsimd writes into each chunk's
            # corner (reading the last gather output) force every weight DMA
            # to dispatch after the gathers, so the small transfers win the
            # DMA queue and the encoder starts immediately. ----
            WCH = 4000
            for w0 in range(0, V, WCH):
                for k in range(KC):
                    nc.gpsimd.tensor_copy(out=w_sb[0:1, k, w0:w0 + 4],
                                          in_=xt[0:1, T - 1, 0, 0:4])
            for w0 in range(0, V, WCH):
                for k in range(KC):
                    nc.sync.dma_start(
                        out=w_sb[:, k, w0:w0 + WCH],
                        in_=wot_d[k * 128:(k + 1) * 128, w0:w0 + WCH])

            # ---- encoder: layer 2 runs one step skewed so each step is ONE
            # activation computing [h1(t), h2(t-1)] — a single PE->ACT->PE
            # round trip per step with no WAR edges. ----
            nc.vector.memset(enc_all[:, 0, 1, :, :], 0.0)  # h2(-1) = 0
            with tc.tile_pool(name="pe1", bufs=2, space="PSUM") as pe1:
                for t in range(T + 1):
                    if t == 0:
                        # h1(0) = tanh(x0) (b1 folded into E)
                        nc.scalar.activation(
                            out=enc_all[:, 0, 0, :, :], in_=xs[:, 0, :, :],
                            func=AF.Tanh)
                        continue
                    bk1 = pe1.tile([128, 2, KC, BL], DT, tag="b1")
                    if t < T:
                        for m in range(KC):
                            for k in range(KC):
                                nc.tensor.matmul(
                                    out=bk1[:, 0, m, :],
                                    lhsT=u_sb[:, k, m * 128:(m + 1) * 128],
                                    rhs=enc_all[:, t - 1, 0, k, :],
                                    start=(m == 0 and k == 0), stop=False)
                        nc.tensor.matmul(
                            out=bk1[:, 0, :, :], lhsT=identb[:, :],
                            rhs=xs[:, t, :, :], start=False, stop=False)
                    for m in range(KC):
                        for k in range(KC):
                            nc.tensor.matmul(
                                out=bk1[:, 1, m, :],
                                lhsT=u_sb[:, k, m * 128:(m + 1) * 128],
                                rhs=enc_all[:, t - 1, 1, k, :],
                                start=(t == T and m == 0 and k == 0),
                                stop=False)
                    nc.tensor.matmul(
                        out=bk1[:, 1, :, :], lhsT=identb[:, :],
                        rhs=b2b_sb[:, :, :], start=False, stop=False)
                    nc.tensor.matmul(
                        out=bk1[:, 1, :, :], lhsT=identb[:, :],
                        rhs=enc_all[:, t - 1, 0, :, :], start=False, stop=True)
                    if t < T:
                        nc.scalar.activation(
                            out=enc_all[:, t, :, :, :], in_=bk1[:, :, :, :],
                            func=AF.Tanh)
                    else:
                        nc.scalar.activation(
                            out=enc_all[:, t, 1, :, :], in_=bk1[:, 1, :, :],
                            func=AF.Tanh)
                    nc.gpsimd.tensor_copy(out=hd_enc[:, :, :, t - 1],
                                          in_=enc_all[:, t, 1, :, :])

            nc.gpsimd.tensor_copy(out=hT_sb[:, :, :], in_=hd_enc[:, :, :, T - 1])

            # ---- decoder + attention + projection (block-pipelined) ----
            dchunks = []
            n0 = 0
            while n0 < V:
                dchunks.append((n0, min(1024, V - n0)))
                n0 += 1024
            with (
                tc.tile_pool(name="pd", bufs=1, space="PSUM") as pd,
                tc.tile_pool(name="pa", bufs=1, space="PSUM") as pa,
                tc.tile_pool(name="pl", bufs=4, space="PSUM") as pl,
                tc.tile_pool(name="lt", bufs=4) as ltp,
                tc.tile_pool(name="aw", bufs=2) as awp,
            ):
                pending = deque()
                pieces = deque()

                def make_attention_pieces(j):
                    """Attention for block j as small thunks, interleaved one
                    per decoder step of block j+1 so the serial softmax chain
                    never head-of-line-blocks projection work in the FIFOs."""
                    t0 = j * TB
                    st = {}

                    def p_scores(b):
                        def f():
                            if b == 0:
                                st["psS"] = pa.tile([TB, BL, 128], DT, tag="s",
                                                    name="psS")
                            psS = st["psS"]
                            for k in range(KC):
                                nc.tensor.matmul(
                                    out=psS[:, b, :],
                                    lhsT=hd_dbt[:, k, b, t0:t0 + TB],
                                    rhs=hd_enc[:, k, b, :],
                                    start=(b == 0 and k == 0), stop=False)
                            nc.tensor.matmul(
                                out=psS[:, b, :], lhsT=ones1[:, :],
                                rhs=maddb[:, b, :],
                                start=False, stop=(b == BL - 1))
                        return f

                    def p_exp():
                        ex = awp.tile([TB, BL, 128], DT, tag="ex")
                        st["ex"] = ex
                        nc.scalar.activation(out=ex[:, :, :],
                                             in_=st["psS"][:, :, :],
                                             func=AF.Exp, scale=1.0 / 16.0)
                        st["alpha"] = awp.tile([TB, BL, 128], BF, tag="alpha",
                                               name="alpha")

                    def p_soft(b):
                        def f():
                            sm = wp.tile([TB, 1], DT, tag="sm")
                            nc.vector.reduce_sum(out=sm[:, :],
                                                 in_=st["ex"][:, b, :],
                                                 axis=AX.X)
                            rs = wp.tile([TB, 1], DT, tag="rs")
                            nc.vector.reciprocal(out=rs[:, :], in_=sm[:, :])
                            nc.vector.tensor_scalar(
                                out=st["alpha"][:, b, :], in0=st["ex"][:, b, :],
                                scalar1=rs[:, :1], scalar2=None, op0=ALU.mult)
                        return f

                    def p_aT(b):
                        def f():
                            if b == 0:
                                st["aT"] = awp.tile([128, BL, TB], BF,
                                                    tag="aT", name="aT")
                            psT = pa.tile([128, TB], DT, tag="t", name="psT")
                            nc.tensor.matmul(out=psT[:, :],
                                             lhsT=st["alpha"][:, b, :],
                                             rhs=identb[0:TB, 0:TB],
                                             start=True, stop=True)
                            nc.vector.tensor_copy(out=st["aT"][:, b, :],
                                                  in_=psT[:, :])
                        return f

                    def p_ctx(b, k):
                        def f():
                            if b == 0 and k == 0:
                                st["ctx"] = awp.tile([128, KC, TB, BL], BF,
                                                     tag="ctx", name="ctx")
                            psC = pa.tile([128, TB], DT, tag="c", name="psC")
                            nc.tensor.matmul(
                                out=psC[:, :], lhsT=ht_enc[:, b, k, :],
                                rhs=st["aT"][:, b, :], start=True, stop=True)
                            nc.vector.tensor_copy(
                                out=st["ctx"][:, k, :, b], in_=psC[:, :])
                        return f

                    def p_out(m):
                        def f():
                            psO = pa.tile([128, TB * BL], DT, tag="s",
                                          name="psO")
                            for k in range(KC):
                                nc.tensor.matmul(
                                    out=psO[:, :],
                                    lhsT=cwt_sb[:, k, m * 128:(m + 1) * 128],
                                    rhs=st["ctx"][:, k, :, :],
                                    start=(k == 0), stop=False)
                            nc.tensor.matmul(
                                out=psO[:, :], lhsT=identb[:, :],
                                rhs=hd_dec[:, m, t0 * BL:t0 * BL + TB * BL],
                                start=False, stop=True)
                            nc.vector.tensor_copy(
                                out=houts[:, m, j * 128:(j + 1) * 128],
                                in_=psO[:, :])
                            if m == KC - 1:
                                pending.extend((j, n0, nv)
                                               for (n0, nv) in dchunks)
                        return f

                    out = [p_scores(b) for b in range(BL)]
                    out.append(p_exp)
                    # per-b softmax->transpose->ctx so outproj unblocks early
                    for b in range(BL):
                        out.append(p_soft(b))
                        out.append(p_aT(b))
                        out.extend(p_ctx(b, k) for k in range(KC))
                    out.extend(p_out(m) for m in range(KC))
                    return out

                # projection: 1024-col double-chunks over four rotating
                # single-bank PSUM tiles; the two half-copies go to DVE and
                # ACT in parallel; one 1024-wide DMA per double-chunk.
                def emit_proj_chunk(j, n0, nv, engine):
                    lt = ltp.tile([128, 1024], BF, tag="lt", name="lt")
                    for h0 in range(0, nv, 512):
                        hv = min(512, nv - h0)
                        plt = pl.tile([128, 512], DT, tag="pl", name="plt")
                        for k in range(KC):
                            nc.tensor.matmul(
                                out=plt[:, :hv],
                                lhsT=houts[:, k, j * 128:(j + 1) * 128],
                                rhs=w_sb[:, k, n0 + h0:n0 + h0 + hv],
                                start=(k == 0), stop=(k == KC - 1))
                        if (h0 == 0) == (engine == "act"):
                            nc.scalar.copy(out=lt[:, h0:h0 + hv],
                                           in_=plt[:, :hv])
                        else:
                            nc.vector.tensor_copy(out=lt[:, h0:h0 + hv],
                                                  in_=plt[:, :hv])
                    nc.sync.dma_start(
                        out=out_d[j * 128:(j + 1) * 128, n0:n0 + nv],
                        in_=lt[:, :nv])

                for t in range(T):
                    bkd = pd.tile([128, KC, BL], DT, tag="d")
                    for m in range(KC):
                        for k in range(KC):
                            rhs = (hT_sb[:, k, :] if t == 0
                                   else hd_dec[:, k, (t - 1) * BL:t * BL])
                            nc.tensor.matmul(
                                out=bkd[:, m, :],
                                lhsT=u_sb[:, k, m * 128:(m + 1) * 128],
                                rhs=rhs, start=(m == 0 and k == 0), stop=False)
                    nc.tensor.matmul(
                        out=bkd[:, :, :], lhsT=identb[:, :],
                        rhs=xt[:, t, :, :], start=False, stop=True)
                    nc.scalar.activation(
                        out=hd_dec[:, :, t * BL:(t + 1) * BL], in_=bkd[:, :, :],
                        func=AF.Tanh)
                    nc.gpsimd.tensor_copy(out=hd_dbt[:, :, :, t],
                                          in_=hd_dec[:, :, t * BL:(t + 1) * BL])
                    # H' -> H^T transposes ride the idle first decoder steps
                    # (attention only needs ht_enc from step TB onwards)
                    if t < BL * KC:
                        b, k = divmod(t, KC)
                        tp2 = pa.tile([128, 128], DT, tag="s", name="tp2")
                        nc.tensor.matmul(
                            out=tp2[:, :], lhsT=hd_enc[:, k, b, :],
                            rhs=identb[:, :], start=True, stop=True)
                        nc.vector.tensor_copy(
                            out=ht_enc[:, b, k, :], in_=tp2[:, :])
                    if t % TB == TB - 1:
                        pieces.extend(make_attention_pieces(t // TB))
                        # kick the scores pieces immediately (they only
                        # need this block's now-complete hidden states)
                        for _ in range(4):
                            pieces.popleft()()
                    if interleave:
                        npop = 2 if len(pending) > 40 else 1
                        for i in range(npop):
                            if pending:
                                eng = ("act" if (t % 2 == 0 and i == 0)
                                       else "dve")
                                emit_proj_chunk(*pending.popleft(), engine=eng)
                    if pieces:
                        pieces.popleft()()
                # drain: finish last block's attention pieces + projection
                i = 0
                while pieces or pending:
                    if pending:
                        eng = "act" if i % 2 == 0 else "dve"
                        emit_proj_chunk(*pending.popleft(), engine=eng)
                        i += 1
                    if pieces:
                        pieces.popleft()()
    nc.compile()
    return nc


def _prep_in_maps(U, b_enc1, b_enc2, b_dec, E_en, E_de, ctx_W, W_out_de,
                  src_en, tgt_de_in):
    f32 = np.float32
    U = np.ascontiguousarray(U, f32).astype(NPBF)
    ctx_wt = np.ascontiguousarray(np.asarray(ctx_W, f32).T).astype(NPBF)
    w_out_t = np.ascontiguousarray(np.asarray(W_out_de, f32).T).astype(NPBF)
    E_en = (np.asarray(E_en, f32) + np.asarray(b_enc1, f32)[None, :]).astype(NPBF)
    E_de = (np.asarray(E_de, f32) + np.asarray(b_dec, f32)[None, :]).astype(NPBF)
    b2 = np.asarray(b_enc2, f32).reshape(KC, 128).T  # [128, KC]
    b2b = np.repeat(b2[:, :, None], BL, axis=2).reshape(128, KC * BL).astype(NPBF)
    src = np.asarray(src_en).astype(np.int32)
    tgt = np.asarray(tgt_de_in).astype(np.int32)
    in_maps = []
    for i in range(NCORES):
        b0 = i * BL
        in_maps.append({
            "u": U, "ctx_wt": ctx_wt, "w_out_t": w_out_t,
            "e_en": E_en, "e_de": E_de, "b2b": b2b,
            "src_idx": np.ascontiguousarray(src[:, b0:b0 + BL]),
            "tgt_idx": np.ascontiguousarray(tgt[:, b0:b0 + BL]),
        })
    return in_maps


def kernel(U, b_enc1, b_enc2, b_dec, E_en, E_de, ctx_W, W_out_de,
           src_en, tgt_de_in, _trace=False, _raw=False, _ncores=NCORES):
    if "nc" not in _CACHE:
        _CACHE["nc"] = _build()
    nc = _CACHE["nc"]
    in_maps = _prep_in_maps(U, b_enc1, b_enc2, b_dec, E_en, E_de, ctx_W,
                            W_out_de, src_en, tgt_de_in)[:_ncores]
    res = run_bass_kernel_spmd(nc, in_maps, list(range(_ncores)), trace=_trace)
    if _raw:
        return res
    logits = np.empty((T, _ncores * BL, V), np.float32)
    for i in range(_ncores):
        logits[:, i * BL:(i + 1) * BL, :] = (
            res.results[i]["out"].astype(np.float32).reshape(T, BL, V))
    if _trace:
        return logits, res
    return logits


# revision 27
# speedup vs baseline: 1.1401x; 1.0131x over previous
"""Seq2seq RNN with attention on 8 TRN2 NeuronCores.

Data-parallel over batch (B=32 -> 4 per core). Structure:

- Attention is hoisted OUT of the decoder loop (the recurrence doesn't
  depend on it) and computed per 32-step block as batched matmuls, split
  into small pieces interleaved one-per-step with the decoder so the
  serial softmax chain never head-of-line-blocks the engine FIFOs.
- The 2-layer encoder runs layer 2 skewed one step behind layer 1: each
  step is ONE activation computing [h1(t), h2(t-1)], written to fresh
  columns of a state-history tile so the chain has no WAR edges — a
  single PE->ACT->PE semaphore round trip (~700ns) per step.
- All elementwise adds are folded into PSUM via identity-matmul
  accumulation; b_enc1/b_dec are folded into the embeddings on the host.
- W_out.T (bf16) is DMA-prefetched into SBUF during the encoder, chunked
  and gated behind the embedding gathers.
- The final projection (107us of PE work) is interleaved with the
  decoder blocks; each 1024-col logit chunk's two halves are copied
  PSUM->SBUF on DVE and ACT in parallel over four rotating single-bank
  PSUM buffers; output DMAs are 1024 columns wide.
"""

import numpy as np
from collections import deque

import concourse.bass as bass
import concourse.bacc as bacc
import concourse.tile as tile
from concourse import mybir
from concourse.bass_utils import run_bass_kernel_spmd
from concourse.masks import make_identity

D = 256
V = 32000
T = 128  # T_SRC == T_TGT == 128
B = 32
NCORES = 8
BL = B // NCORES  # 4 batch elements per core
KC = D // 128  # 2 d-chunks of 128
TB = 32  # decoder block size
NBLK = T // TB
DT = mybir.dt.float32
BF = mybir.dt.bfloat16
NPBF = mybir.dt.np(BF)
AF = mybir.ActivationFunctionType
ALU = mybir.AluOpType
AX = mybir.AxisListType

_CACHE = {}

# projection column chunks: pairs that share one 1024-wide output DMA
def _nchunks():
    out = []
    n0 = 0
    while n0 < V:
        out.append((n0, min(512, V - n0)))
        n0 += 512
    return out


def _build(interleave=True):
    nc = bacc.Bacc(None)

    u_d = nc.declare_dram_parameter("u", [D, D], BF, isOutput=False)
    cwt_d = nc.declare_dram_parameter("ctx_wt", [D, D], BF, isOutput=False)
    wot_d = nc.declare_dram_parameter("w_out_t", [D, V], BF, isOutput=False)
    een_d = nc.declare_dram_parameter("e_en", [V, D], BF, isOutput=False)
    ede_d = nc.declare_dram_parameter("e_de", [V, D], BF, isOutput=False)
    b2b_d = nc.declare_dram_parameter("b2b", [128, KC * BL], BF, isOutput=False)
    si_d = nc.declare_dram_parameter("src_idx", [T, BL], mybir.dt.int32, isOutput=False)
    ti_d = nc.declare_dram_parameter("tgt_idx", [T, BL], mybir.dt.int32, isOutput=False)
    out_d = nc.declare_dram_parameter("out", [T * BL, V], BF, isOutput=True)

    with tile.TileContext(nc) as tc:
        with (
            tc.tile_pool(name="persist", bufs=1) as pp,
            tc.tile_pool(name="work", bufs=4) as wp,
        ):
            # ---- persistent SBUF tiles ----
            u_sb = pp.tile([128, KC, D], BF, tag="u")
            cwt_sb = pp.tile([128, KC, D], BF, tag="cwt")
            w_sb = pp.tile([128, KC, V], BF, tag="w")  # W_out.T chunks
            ident = pp.tile([128, 128], DT, tag="ident")
            identb = pp.tile([128, 128], BF, tag="identb")
            ones1 = pp.tile([1, TB], BF, tag="ones1")
            b2b_sb = pp.tile([128, KC, BL], BF, tag="b2b")
            si_sb = pp.tile([T, BL], mybir.dt.int32, tag="si")
            ti_sb = pp.tile([T, BL], mybir.dt.int32, tag="ti")
            maddb = pp.tile([1, BL, T], BF, tag="maddb")  # -1e9 at PAD, on part 0
            xs = pp.tile([128, T, KC, BL], BF, tag="xs")  # x_src' [d, t, m, b]
            xt = pp.tile([128, T, KC, BL], BF, tag="xt")  # x_tgt'
            hd_enc = pp.tile([128, KC, BL, T], BF, tag="hd")  # H' [d,k,b,t]
            ht_enc = pp.tile([128, BL, KC, 128], BF, tag="ht")  # H^T [t,b,k,d]
            # encoder state history: slot [t,0]=h1(t), [t,1]=h2(t-1) — every
            # activation writes FRESH columns, so there is no WAR edge on the
            # recurrence chain (matches the decoder's 652ns/step rhythm)
            enc_all = pp.tile([128, T + 1, KC, 2, BL], BF, tag="enc_all")
            hT_sb = pp.tile([128, KC, BL], BF, tag="hT")  # enc final h2
            hd_dec = pp.tile([128, KC, T * BL], BF, tag="hdd")  # dec h', r=t*BL+b
            hd_dbt = pp.tile([128, KC, BL, T], BF, tag="hdbt")  # dec h' [d,k,b,t]
            houts = pp.tile([128, KC, T * BL], BF, tag="houts")  # outs'

            # ---- small constant loads. si first (gathers depend on it),
            # then u/cwt/b2b (needed at encoder step 1, ~8us in; their
            # transfers are tiny and barely delay the gathers) ----
            nc.sync.dma_start(out=si_sb[:, :], in_=si_d[:, :])
            nc.sync.dma_start(out=ti_sb[:, :], in_=ti_d[:, :])
            for k in range(KC):
                nc.sync.dma_start(out=u_sb[:, k, :], in_=u_d[k * 128:(k + 1) * 128, :])
                nc.sync.dma_start(out=cwt_sb[:, k, :], in_=cwt_d[k * 128:(k + 1) * 128, :])
            nc.sync.dma_start(out=b2b_sb[:, :, :], in_=b2b_d[:, :])
            make_identity(nc, ident[:, :])
            nc.vector.tensor_copy(out=identb[:, :], in_=ident[:, :])
            nc.vector.memset(ones1[:, :], 1.0)
            nc.vector.memset(enc_all[:, 0, :, 1, :], 0.0)  # h2(-1) = 0
            # dummy activation: pulls the ~2.7us ACT table load (tanh/exp
            # share one set) into the setup phase instead of encoder t=0
            warm = wp.tile([1, 1], DT, tag="warm")
            nc.scalar.activation(out=warm[:, :], in_=ident[0:1, 0:1],
                                 func=AF.Tanh)

            _xgd_tiles = []
            # ---- src embedding gathers + transposes (encoder-critical).
            # The tgt gathers/transposes and the attention mask are emitted
            # inside the encoder loop so the scheduler keeps them off the
            # encoder's critical-path PE slots. ----
            with tc.tile_pool(name="pst", bufs=4, space="PSUM") as pst:
                for b in range(BL):
                    xg = wp.tile([T, D], BF, tag="xg", bufs=8)
                    nc.gpsimd.indirect_dma_start(
                        out=xg[:, :],
                        out_offset=None,
                        in_=een_d[:, :],
                        in_offset=bass.IndirectOffsetOnAxis(
                            ap=si_sb[:, b:b + 1], axis=0),
                    )
                    for k in range(KC):
                        tp = pst.tile([128, 128], DT, tag="tp")
                        nc.tensor.matmul(
                            out=tp[:, :], lhsT=xg[:, k * 128:(k + 1) * 128],
                            rhs=identb[:, :], start=True, stop=True)
                        nc.vector.tensor_copy(out=xs[:, :, k, b], in_=tp[:, :])
                for b in range(BL):
                    xgd = wp.tile([T, D], BF, tag="xgd", bufs=4)
                    nc.gpsimd.indirect_dma_start(
                        out=xgd[:, :],
                        out_offset=None,
                        in_=ede_d[:, :],
                        in_offset=bass.IndirectOffsetOnAxis(
                            ap=ti_sb[:, b:b + 1], axis=0),
                    )
                    _xgd_tiles.append(xgd)

            # ---- big weight prefetch. Tiny gpsimd writes into each chunk's
            # corner (reading the last gather output) force every weight DMA
            # to dispatch after the gathers, so the small transfers win the
            # DMA queue and the encoder starts immediately. ----
            WCH = 4000
            for w0 in range(0, V, WCH):
                for k in range(KC):
                    nc.gpsimd.tensor_copy(out=w_sb[0:1, k, w0:w0 + 4],
                                          in_=xs[0:1, T - 1, 0, 0:4])
            for w0 in range(0, V, WCH):
                for k in range(KC):
                    nc.sync.dma_start(
                        out=w_sb[:, k, w0:w0 + WCH],
                        in_=wot_d[k * 128:(k + 1) * 128, w0:w0 + WCH])

            # ---- encoder: layer 2 runs one step skewed so each step is ONE
            # activation computing [h1(t), h2(t-1)] — a single PE->ACT->PE
            # round trip per step with no WAR edges. ----
            _mask_t = [None]
            with tc.tile_pool(name="pe1", bufs=2, space="PSUM") as pe1:
                for t in range(T + 1):
                    if t == 0:
                        # h1(0) = tanh(x0) (b1 folded into E)
                        nc.scalar.activation(
                            out=enc_all[:, 0, :, 0, :], in_=xs[:, 0, :, :],
                            func=AF.Tanh)
                        continue
                    bk1 = pe1.tile([128, KC, 2, BL], DT, tag="b1")
                    if t < T:
                        # one matmul per (m,k) computes BOTH layers' U
                        # products: rhs = [h1(t-1) | h2(t-2)] for chunk k
                        for m in range(KC):
                            for k in range(KC):
                                nc.tensor.matmul(
                                    out=bk1[:, m, :, :],
                                    lhsT=u_sb[:, k, m * 128:(m + 1) * 128],
                                    rhs=enc_all[:, t - 1, k, :, :],
                                    start=(m == 0 and k == 0), stop=False)
                        for m in range(KC):
                            nc.tensor.matmul(
                                out=bk1[:, m, 0, :], lhsT=identb[:, :],
                                rhs=xs[:, t, m, :], start=False, stop=False)
                    else:
                        for m in range(KC):
                            for k in range(KC):
                                nc.tensor.matmul(
                                    out=bk1[:, m, 1, :],
                                    lhsT=u_sb[:, k, m * 128:(m + 1) * 128],
                                    rhs=enc_all[:, t - 1, k, 1, :],
                                    start=(m == 0 and k == 0), stop=False)
                    for m in range(KC):
                        nc.tensor.matmul(
                            out=bk1[:, m, 1, :], lhsT=identb[:, :],
                            rhs=b2b_sb[:, m, :], start=False, stop=False)
                        nc.tensor.matmul(
                            out=bk1[:, m, 1, :], lhsT=identb[:, :],
                            rhs=enc_all[:, t - 1, m, 0, :], start=False,
                            stop=(m == KC - 1))
                    if t < T:
                        nc.scalar.activation(
                            out=enc_all[:, t, :, :, :], in_=bk1[:, :, :, :],
                            func=AF.Tanh)
                    else:
                        nc.scalar.activation(
                            out=enc_all[:, t, :, 1, :], in_=bk1[:, :, 1, :],
                            func=AF.Tanh)
                    nc.gpsimd.tensor_copy(out=hd_enc[:, :, :, t - 1],
                                          in_=enc_all[:, t, :, 1, :])
                    # deferred setup work rides the encoder's idle PE/DVE
                    # slots — placed late enough (t>=20) that the gather
                    # transfers have landed, so the in-order PE queue never
                    # stalls on them: tgt-embedding transposes (needed at
                    # decoder t=0) then the attention mask (decoder t=32)
                    if 20 <= t < 20 + BL * KC:
                        b, k = divmod(t - 20, KC)
                        tp = pe1.tile([128, 128], DT, tag="tp", bufs=2)
                        nc.tensor.matmul(
                            out=tp[:, :],
                            lhsT=_xgd_tiles[b][:, k * 128:(k + 1) * 128],
                            rhs=identb[:, :], start=True, stop=True)
                        nc.vector.tensor_copy(out=xt[:, :, k, b], in_=tp[:, :])
                    elif t == 20 + BL * KC:
                        mf = wp.tile([T, BL], BF, tag="mf")
                        nc.vector.tensor_copy(out=mf[:, :], in_=si_sb[:, :])
                        m01 = wp.tile([T, BL], BF, tag="m01")
                        nc.vector.tensor_scalar(
                            out=m01[:, :], in0=mf[:, :], scalar1=0.0,
                            scalar2=None, op0=ALU.is_equal,
                        )
                        _mask_t[0] = m01
                    elif 21 + BL * KC <= t < 21 + BL * KC + BL:
                        b = t - (21 + BL * KC)
                        psM = pe1.tile([1, T], DT, tag="psM", bufs=2)
                        nc.tensor.matmul(out=psM[:, :],
                                         lhsT=_mask_t[0][:, b:b + 1],
                                         rhs=identb[:, :], start=True,
                                         stop=True)
                        nc.vector.tensor_scalar(
                            out=maddb[:, b, :], in0=psM[:, :], scalar1=-1e9,
                            scalar2=None, op0=ALU.mult,
                        )

            nc.gpsimd.tensor_copy(out=hT_sb[:, :, :], in_=hd_enc[:, :, :, T - 1])

            # ---- decoder + attention + projection (block-pipelined) ----
            dchunks = []
            n0 = 0
            while n0 < V:
                dchunks.append((n0, min(1024, V - n0)))
                n0 += 1024
            with (
                tc.tile_pool(name="pd", bufs=1, space="PSUM") as pd,
                tc.tile_pool(name="pa", bufs=1, space="PSUM") as pa,
                tc.tile_pool(name="pl", bufs=4, space="PSUM") as pl,
                tc.tile_pool(name="lt", bufs=4) as ltp,
                tc.tile_pool(name="aw", bufs=2) as awp,
            ):
                pending = deque()
                pieces = deque()

                def make_attention_pieces(j):
                    """Attention for block j as small thunks, interleaved one
                    per decoder step of block j+1 so the serial softmax chain
                    never head-of-line-blocks projection work in the FIFOs."""
                    t0 = j * TB
                    st = {}

                    def p_scores(b):
                        def f():
                            if b == 0:
                                st["psS"] = pa.tile([TB, BL, 128], DT, tag="s",
                                                    name="psS")
                            psS = st["psS"]
                            for k in range(KC):
                                nc.tensor.matmul(
                                    out=psS[:, b, :],
                                    lhsT=hd_dbt[:, k, b, t0:t0 + TB],
                                    rhs=hd_enc[:, k, b, :],
                                    start=(b == 0 and k == 0), stop=False)
                            nc.tensor.matmul(
                                out=psS[:, b, :], lhsT=ones1[:, :],
                                rhs=maddb[:, b, :],
                                start=False, stop=(b == BL - 1))
                        return f

                    def p_exp():
                        ex = awp.tile([TB, BL, 128], DT, tag="ex")
                        st["ex"] = ex
                        nc.scalar.activation(out=ex[:, :, :],
                                             in_=st["psS"][:, :, :],
                                             func=AF.Exp, scale=1.0 / 16.0)
                        st["alpha"] = awp.tile([TB, BL, 128], BF, tag="alpha",
                                               name="alpha")

                    def p_soft(b):
                        def f():
                            sm = wp.tile([TB, 1], DT, tag="sm")
                            nc.vector.reduce_sum(out=sm[:, :],
                                                 in_=st["ex"][:, b, :],
                                                 axis=AX.X)
                            rs = wp.tile([TB, 1], DT, tag="rs")
                            nc.vector.reciprocal(out=rs[:, :], in_=sm[:, :])
                            nc.vector.tensor_scalar(
                                out=st["alpha"][:, b, :], in0=st["ex"][:, b, :],
                                scalar1=rs[:, :1], scalar2=None, op0=ALU.mult)
                        return f

                    def p_aT(b):
                        def f():
                            if b == 0:
                                st["aT"] = awp.tile([128, BL, TB], BF,
                                                    tag="aT", name="aT")
                            psT = pa.tile([128, TB], DT, tag="t", name="psT")
                            nc.tensor.matmul(out=psT[:, :],
                                             lhsT=st["alpha"][:, b, :],
                                             rhs=identb[0:TB, 0:TB],
                                             start=True, stop=True)
                            nc.vector.tensor_copy(out=st["aT"][:, b, :],
                                                  in_=psT[:, :])
                        return f

                    def p_ctx(b, k):
                        def f():
                            if b == 0 and k == 0:
                                st["ctx"] = awp.tile([128, KC, TB, BL], BF,
                                                     tag="ctx", name="ctx")
                            psC = pa.tile([128, TB], DT, tag="c", name="psC")
                            nc.tensor.matmul(
                                out=psC[:, :], lhsT=ht_enc[:, b, k, :],
                                rhs=st["aT"][:, b, :], start=True, stop=True)
                            nc.vector.tensor_copy(
                                out=st["ctx"][:, k, :, b], in_=psC[:, :])
                        return f

                    def p_out(m):
                        def f():
                            psO = pa.tile([128, TB * BL], DT, tag="s",
                                          name="psO")
                            for k in range(KC):
                                nc.tensor.matmul(
                                    out=psO[:, :],
                                    lhsT=cwt_sb[:, k, m * 128:(m + 1) * 128],
                                    rhs=st["ctx"][:, k, :, :],
                                    start=(k == 0), stop=False)
                            nc.tensor.matmul(
                                out=psO[:, :], lhsT=identb[:, :],
                                rhs=hd_dec[:, m, t0 * BL:t0 * BL + TB * BL],
                                start=False, stop=True)
                            nc.vector.tensor_copy(
                                out=houts[:, m, j * 128:(j + 1) * 128],
                                in_=psO[:, :])
                            if m == KC - 1:
                                pending.extend((j, n0, nv)
                                               for (n0, nv) in dchunks)
                        return f

                    out = [p_scores(b) for b in range(BL)]
                    out.append(p_exp)
                    # per-b softmax->transpose->ctx so outproj unblocks early
                    for b in range(BL):
                        out.append(p_soft(b))
                        out.append(p_aT(b))
                        out.extend(p_ctx(b, k) for k in range(KC))
                    out.extend(p_out(m) for m in range(KC))
                    return out

                # projection: 1024-col double-chunks over four rotating
                # single-bank PSUM tiles; the two half-copies go to DVE and
                # ACT in parallel; one 1024-wide DMA per double-chunk.
                def emit_proj_chunk(j, n0, nv, engine):
                    lt = ltp.tile([128, 1024], BF, tag="lt", name="lt")
                    for h0 in range(0, nv, 512):
                        hv = min(512, nv - h0)
                        plt = pl.tile([128, 512], DT, tag="pl", name="plt")
                        for k in range(KC):
                            nc.tensor.matmul(
                                out=plt[:, :hv],
                                lhsT=houts[:, k, j * 128:(j + 1) * 128],
                                rhs=w_sb[:, k, n0 + h0:n0 + h0 + hv],
                                start=(k == 0), stop=(k == KC - 1))
                        if (h0 == 0) == (engine == "act"):
                            nc.scalar.copy(out=lt[:, h0:h0 + hv],
                                           in_=plt[:, :hv])
                        else:
                            nc.vector.tensor_copy(out=lt[:, h0:h0 + hv],
                                                  in_=plt[:, :hv])
                    nc.sync.dma_start(
                        out=out_d[j * 128:(j + 1) * 128, n0:n0 + nv],
                        in_=lt[:, :nv])

                for t in range(T):
                    bkd = pd.tile([128, KC, BL], DT, tag="d")
                    for m in range(KC):
                        for k in range(KC):
                            rhs = (hT_sb[:, k, :] if t == 0
                                   else hd_dec[:, k, (t - 1) * BL:t * BL])
                            nc.tensor.matmul(
                                out=bkd[:, m, :],
                                lhsT=u_sb[:, k, m * 128:(m + 1) * 128],
                                rhs=rhs, start=(m == 0 and k == 0), stop=False)
                    nc.tensor.matmul(
                        out=bkd[:, :, :], lhsT=identb[:, :],
                        rhs=xt[:, t, :, :], start=False, stop=True)
                    nc.scalar.activation(
                        out=hd_dec[:, :, t * BL:(t + 1) * BL], in_=bkd[:, :, :],
                        func=AF.Tanh)
                    nc.gpsimd.tensor_copy(out=hd_dbt[:, :, :, t],
                                          in_=hd_dec[:, :, t * BL:(t + 1) * BL])
                    # H' -> H^T transposes ride the idle first decoder steps
                    # (attention only needs ht_enc from step TB onwards)
                    if t < BL * KC:
                        b, k = divmod(t, KC)
                        tp2 = pa.tile([128, 128], DT, tag="s", name="tp2")
                        nc.tensor.matmul(
                            out=tp2[:, :], lhsT=hd_enc[:, k, b, :],
                            rhs=identb[:, :], start=True, stop=True)
                        nc.vector.tensor_copy(
                            out=ht_enc[:, b, k, :], in_=tp2[:, :])
                    if t % TB == TB - 1:
                        pieces.extend(make_attention_pieces(t // TB))
                        # kick the scores pieces immediately (they only
                        # need this block's now-complete hidden states)
                        for _ in range(4):
                            pieces.popleft()()
                    if interleave:
                        npop = 2 if len(pending) > 40 else 1
                        for i in range(npop):
                            if pending:
                                eng = ("act" if (t % 2 == 0 and i == 0)
                                       else "dve")
                                emit_proj_chunk(*pending.popleft(), engine=eng)
                    if pieces:
                        pieces.popleft()()
                # drain: finish last block's attention pieces + projection
                i = 0
                while pieces or pending:
                    if pending:
                        eng = "act" if i % 2 == 0 else "dve"
                        emit_proj_chunk(*pending.popleft(), engine=eng)
                        i += 1
                    if pieces:
                        pieces.popleft()()
    nc.compile()
    return nc


def _prep_in_maps(U, b_enc1, b_enc2, b_dec, E_en, E_de, ctx_W, W_out_de,
                  src_en, tgt_de_in):
    f32 = np.float32
    U = np.ascontiguousarray(U, f32).astype(NPBF)
    ctx_wt = np.ascontiguousarray(np.asarray(ctx_W, f32).T).astype(NPBF)
    w_out_t = np.ascontiguousarray(np.asarray(W_out_de, f32).T).astype(NPBF)
    E_en = (np.asarray(E_en, f32) + np.asarray(b_enc1, f32)[None, :]).astype(NPBF)
    E_de = (np.asarray(E_de, f32) + np.asarray(b_dec, f32)[None, :]).astype(NPBF)
    b2 = np.asarray(b_enc2, f32).reshape(KC, 128).T  # [128, KC]
    b2b = np.repeat(b2[:, :, None], BL, axis=2).reshape(128, KC * BL).astype(NPBF)
    src = np.asarray(src_en).astype(np.int32)
    tgt = np.asarray(tgt_de_in).astype(np.int32)
    in_maps = []
    for i in range(NCORES):
        b0 = i * BL
        in_maps.append({
            "u": U, "ctx_wt": ctx_wt, "w_out_t": w_out_t,
            "e_en": E_en, "e_de": E_de, "b2b": b2b,
            "src_idx": np.ascontiguousarray(src[:, b0:b0 + BL]),
            "tgt_idx": np.ascontiguousarray(tgt[:, b0:b0 + BL]),
        })
    return in_maps


def kernel(U, b_enc1, b_enc2, b_dec, E_en, E_de, ctx_W, W_out_de,
           src_en, tgt_de_in, _trace=False, _raw=False, _ncores=NCORES):
    if "nc" not in _CACHE:
        _CACHE["nc"] = _build()
    nc = _CACHE["nc"]
    in_maps = _prep_in_maps(U, b_enc1, b_enc2, b_dec, E_en, E_de, ctx_W,
                            W_out_de, src_en, tgt_de_in)[:_ncores]
    res = run_bass_kernel_spmd(nc, in_maps, list(range(_ncores)), trace=_trace)
    if _raw:
        return res
    logits = np.empty((T, _ncores * BL, V), np.float32)
    for i in range(_ncores):
        logits[:, i * BL:(i + 1) * BL, :] = (
            res.results[i]["out"].astype(np.float32).reshape(T, BL, V))
    if _trace:
        return logits, res
    return logits


# revision 28
# speedup vs baseline: 1.1827x; 1.0374x over previous
"""Seq2seq RNN with attention on 8 TRN2 NeuronCores.

Data-parallel over batch (B=32 -> 4 per core). Structure:

- Attention is hoisted OUT of the decoder loop (the recurrence doesn't
  depend on it) and computed per 32-step block as batched matmuls, split
  into small pieces interleaved one-per-step with the decoder so the
  serial softmax chain never head-of-line-blocks the engine FIFOs.
- The 2-layer encoder runs layer 2 skewed one step behind layer 1: each
  step is ONE activation computing [h1(t), h2(t-1)], written to fresh
  columns of a state-history tile so the chain has no WAR edges — a
  single PE->ACT->PE semaphore round trip (~700ns) per step.
- All elementwise adds are folded into PSUM via identity-matmul
  accumulation; b_enc1/b_dec are folded into the embeddings on the host.
- W_out.T (bf16) is DMA-prefetched into SBUF during the encoder, chunked
  and gated behind the embedding gathers.
- The final projection (107us of PE work) is interleaved with the
  decoder blocks; each 1024-col logit chunk's two halves are copied
  PSUM->SBUF on DVE and ACT in parallel over four rotating single-bank
  PSUM buffers; output DMAs are 1024 columns wide.
"""

import numpy as np
from collections import deque

import concourse.bass as bass
import concourse.bacc as bacc
import concourse.tile as tile
from concourse import mybir
from concourse.bass_utils import run_bass_kernel_spmd
from concourse.masks import make_identity

D = 256
V = 32000
T = 128  # T_SRC == T_TGT == 128
B = 32
NCORES = 8
BL = B // NCORES  # 4 batch elements per core
KC = D // 128  # 2 d-chunks of 128
TB = 32  # decoder block size
NBLK = T // TB
DT = mybir.dt.float32
BF = mybir.dt.bfloat16
NPBF = mybir.dt.np(BF)
AF = mybir.ActivationFunctionType
ALU = mybir.AluOpType
AX = mybir.AxisListType

_CACHE = {}

# projection column chunks: pairs that share one 1024-wide output DMA
def _nchunks():
    out = []
    n0 = 0
    while n0 < V:
        out.append((n0, min(512, V - n0)))
        n0 += 512
    return out


def _build(interleave=True):
    nc = bacc.Bacc(None)

    u_d = nc.declare_dram_parameter("u", [D, D], BF, isOutput=False)
    cwt_d = nc.declare_dram_parameter("ctx_wt", [D, D], BF, isOutput=False)
    wot_d = nc.declare_dram_parameter("w_out_t", [D, V], BF, isOutput=False)
    een_d = nc.declare_dram_parameter("e_en", [V, D], BF, isOutput=False)
    ede_d = nc.declare_dram_parameter("e_de", [V, D], BF, isOutput=False)
    b2b_d = nc.declare_dram_parameter("b2b", [128, KC * BL], BF, isOutput=False)
    si_d = nc.declare_dram_parameter("src_idx", [T, BL], mybir.dt.int32, isOutput=False)
    ti_d = nc.declare_dram_parameter("tgt_idx", [T, BL], mybir.dt.int32, isOutput=False)
    out_d = nc.declare_dram_parameter("out", [T * BL, V], BF, isOutput=True)

    with tile.TileContext(nc) as tc:
        with (
            tc.tile_pool(name="persist", bufs=1) as pp,
            tc.tile_pool(name="work", bufs=4) as wp,
        ):
            # ---- persistent SBUF tiles ----
            u_sb = pp.tile([128, KC, D], BF, tag="u")
            cwt_sb = pp.tile([128, KC, D], BF, tag="cwt")
            w_sb = pp.tile([128, KC, V], BF, tag="w")  # W_out.T chunks
            ident = pp.tile([128, 128], DT, tag="ident")
            identb = pp.tile([128, 128], BF, tag="identb")
            ones1 = pp.tile([1, TB], BF, tag="ones1")
            b2b_sb = pp.tile([128, KC, BL], BF, tag="b2b")
            si_sb = pp.tile([T, BL], mybir.dt.int32, tag="si")
            ti_sb = pp.tile([T, BL], mybir.dt.int32, tag="ti")
            maddb = pp.tile([1, BL, T], BF, tag="maddb")  # -1e9 at PAD, on part 0
            xs = pp.tile([128, T, KC, BL], BF, tag="xs")  # x_src' [d, t, m, b]
            xt = pp.tile([128, T, KC, BL], BF, tag="xt")  # x_tgt'
            hd_enc = pp.tile([128, KC, BL, T], BF, tag="hd")  # H' [d,k,b,t]
            ht_enc = pp.tile([128, BL, KC, 128], BF, tag="ht")  # H^T [t,b,k,d]
            # encoder state history: slot [t,0]=h1(t), [t,1]=h2(t-1) — every
            # activation writes FRESH columns, so there is no WAR edge on the
            # recurrence chain (matches the decoder's 652ns/step rhythm)
            enc_all = pp.tile([128, T + 1, KC, 2, BL], BF, tag="enc_all")
            hT_sb = pp.tile([128, KC, BL], BF, tag="hT")  # enc final h2
            hd_dec = pp.tile([128, KC, T * BL], BF, tag="hdd")  # dec h', r=t*BL+b
            hd_dbt = pp.tile([128, KC, BL, T], BF, tag="hdbt")  # dec h' [d,k,b,t]
            houts = pp.tile([128, KC, T * BL], BF, tag="houts")  # outs'

            # ---- small constant loads. si first (gathers depend on it),
            # then u/cwt/b2b (needed at encoder step 1, ~8us in; their
            # transfers are tiny and barely delay the gathers) ----
            nc.sync.dma_start(out=si_sb[:, :], in_=si_d[:, :])
            nc.sync.dma_start(out=ti_sb[:, :], in_=ti_d[:, :])
            for k in range(KC):
                nc.sync.dma_start(out=u_sb[:, k, :], in_=u_d[k * 128:(k + 1) * 128, :])
                nc.sync.dma_start(out=cwt_sb[:, k, :], in_=cwt_d[k * 128:(k + 1) * 128, :])
            nc.sync.dma_start(out=b2b_sb[:, :, :], in_=b2b_d[:, :])
            make_identity(nc, ident[:, :])
            nc.vector.tensor_copy(out=identb[:, :], in_=ident[:, :])
            nc.vector.memset(ones1[:, :], 1.0)
            nc.vector.memset(enc_all[:, 0, :, 1, :], 0.0)  # h2(-1) = 0
            # dummy activation: pulls the ~2.7us ACT table load (tanh/exp
            # share one set) into the setup phase instead of encoder t=0
            warm = wp.tile([1, 1], DT, tag="warm")
            nc.scalar.activation(out=warm[:, :], in_=ident[0:1, 0:1],
                                 func=AF.Tanh)

            _xgd_tiles = []
            # ---- src embedding gathers + transposes (encoder-critical).
            # The tgt gathers/transposes and the attention mask are emitted
            # inside the encoder loop so the scheduler keeps them off the
            # encoder's critical-path PE slots. ----
            with tc.tile_pool(name="pst", bufs=4, space="PSUM") as pst:
                for b in range(BL):
                    xg = wp.tile([T, D], BF, tag="xg", bufs=8)
                    nc.gpsimd.indirect_dma_start(
                        out=xg[:, :],
                        out_offset=None,
                        in_=een_d[:, :],
                        in_offset=bass.IndirectOffsetOnAxis(
                            ap=si_sb[:, b:b + 1], axis=0),
                    )
                    for k in range(KC):
                        tp = pst.tile([128, 128], DT, tag="tp")
                        nc.tensor.matmul(
                            out=tp[:, :], lhsT=xg[:, k * 128:(k + 1) * 128],
                            rhs=identb[:, :], start=True, stop=True)
                        nc.vector.tensor_copy(out=xs[:, :, k, b], in_=tp[:, :])
                for b in range(BL):
                    xgd = wp.tile([T, D], BF, tag="xgd", bufs=4)
                    nc.gpsimd.indirect_dma_start(
                        out=xgd[:, :],
                        out_offset=None,
                        in_=ede_d[:, :],
                        in_offset=bass.IndirectOffsetOnAxis(
                            ap=ti_sb[:, b:b + 1], axis=0),
                    )
                    _xgd_tiles.append(xgd)

            # ---- big weight prefetch. Tiny gpsimd writes into each chunk's
            # corner (reading the last gather output) force every weight DMA
            # to dispatch after the gathers, so the small transfers win the
            # DMA queue and the encoder starts immediately. ----
            WCH = 4000
            for w0 in range(0, V, WCH):
                for k in range(KC):
                    nc.gpsimd.tensor_copy(out=w_sb[0:1, k, w0:w0 + 4],
                                          in_=xs[0:1, T - 1, 0, 0:4])
            for w0 in range(0, V, WCH):
                for k in range(KC):
                    nc.sync.dma_start(
                        out=w_sb[:, k, w0:w0 + WCH],
                        in_=wot_d[k * 128:(k + 1) * 128, w0:w0 + WCH])

            # ---- encoder: layer 2 runs one step skewed so each step is ONE
            # activation computing [h1(t), h2(t-1)] — a single PE->ACT->PE
            # round trip per step with no WAR edges. ----
            _mask_t = [None]
            with tc.tile_pool(name="pe1", bufs=2, space="PSUM") as pe1:
                for t in range(T + 1):
                    if t == 0:
                        # h1(0) = tanh(x0) (b1 folded into E)
                        nc.scalar.activation(
                            out=enc_all[:, 0, :, 0, :], in_=xs[:, 0, :, :],
                            func=AF.Tanh)
                        continue
                    bk1 = pe1.tile([128, KC, 2, BL], DT, tag="b1")
                    if t < T:
                        # one matmul per (m,k) computes BOTH layers' U
                        # products: rhs = [h1(t-1) | h2(t-2)] for chunk k
                        for m in range(KC):
                            for k in range(KC):
                                nc.tensor.matmul(
                                    out=bk1[:, m, :, :],
                                    lhsT=u_sb[:, k, m * 128:(m + 1) * 128],
                                    rhs=enc_all[:, t - 1, k, :, :],
                                    start=(m == 0 and k == 0), stop=False)
                        for m in range(KC):
                            nc.tensor.matmul(
                                out=bk1[:, m, 0, :], lhsT=identb[:, :],
                                rhs=xs[:, t, m, :], start=False, stop=False)
                    else:
                        for m in range(KC):
                            for k in range(KC):
                                nc.tensor.matmul(
                                    out=bk1[:, m, 1, :],
                                    lhsT=u_sb[:, k, m * 128:(m + 1) * 128],
                                    rhs=enc_all[:, t - 1, k, 1, :],
                                    start=(m == 0 and k == 0), stop=False)
                    for m in range(KC):
                        nc.tensor.matmul(
                            out=bk1[:, m, 1, :], lhsT=identb[:, :],
                            rhs=b2b_sb[:, m, :], start=False, stop=False)
                        nc.tensor.matmul(
                            out=bk1[:, m, 1, :], lhsT=identb[:, :],
                            rhs=enc_all[:, t - 1, m, 0, :], start=False,
                            stop=(m == KC - 1))
                    if t < T:
                        nc.scalar.activation(
                            out=enc_all[:, t, :, :, :], in_=bk1[:, :, :, :],
                            func=AF.Tanh)
                    else:
                        nc.scalar.activation(
                            out=enc_all[:, t, :, 1, :], in_=bk1[:, :, 1, :],
                            func=AF.Tanh)
                    nc.gpsimd.tensor_copy(out=hd_enc[:, :, :, t - 1],
                                          in_=enc_all[:, t, :, 1, :])
                    # deferred setup work rides the encoder's idle PE/DVE
                    # slots — placed late enough (t>=20) that the gather
                    # transfers have landed, so the in-order PE queue never
                    # stalls on them: tgt-embedding transposes (needed at
                    # decoder t=0) then the attention mask (decoder t=32)
                    if 20 <= t < 20 + BL * KC:
                        b, k = divmod(t - 20, KC)
                        tp = pe1.tile([128, 128], DT, tag="tp", bufs=2)
                        nc.tensor.matmul(
                            out=tp[:, :],
                            lhsT=_xgd_tiles[b][:, k * 128:(k + 1) * 128],
                            rhs=identb[:, :], start=True, stop=True)
                        nc.vector.tensor_copy(out=xt[:, :, k, b], in_=tp[:, :])
                    elif t == 20 + BL * KC:
                        mf = wp.tile([T, BL], BF, tag="mf")
                        nc.vector.tensor_copy(out=mf[:, :], in_=si_sb[:, :])
                        m01 = wp.tile([T, BL], BF, tag="m01")
                        nc.vector.tensor_scalar(
                            out=m01[:, :], in0=mf[:, :], scalar1=0.0,
                            scalar2=None, op0=ALU.is_equal,
                        )
                        _mask_t[0] = m01
                    elif 21 + BL * KC <= t < 21 + BL * KC + BL:
                        b = t - (21 + BL * KC)
                        psM = pe1.tile([1, T], DT, tag="psM", bufs=2)
                        nc.tensor.matmul(out=psM[:, :],
                                         lhsT=_mask_t[0][:, b:b + 1],
                                         rhs=identb[:, :], start=True,
                                         stop=True)
                        nc.vector.tensor_scalar(
                            out=maddb[:, b, :], in0=psM[:, :], scalar1=-1e9,
                            scalar2=None, op0=ALU.mult,
                        )

            nc.gpsimd.tensor_copy(out=hT_sb[:, :, :], in_=hd_enc[:, :, :, T - 1])

            # ---- decoder + attention + projection (block-pipelined) ----
            dchunks = []
            n0 = 0
            while n0 < V:
                dchunks.append((n0, min(1024, V - n0)))
                n0 += 1024
            with (
                tc.tile_pool(name="pd", bufs=1, space="PSUM") as pd,
                tc.tile_pool(name="pa", bufs=1, space="PSUM") as pa,
                tc.tile_pool(name="pl", bufs=4, space="PSUM") as pl,
                tc.tile_pool(name="lt", bufs=10) as ltp,
                tc.tile_pool(name="aw", bufs=2) as awp,
            ):
                pending = deque()
                pieces = deque()

                def make_attention_pieces(j):
                    """Attention for block j as small thunks, interleaved one
                    per decoder step of block j+1 so the serial softmax chain
                    never head-of-line-blocks projection work in the FIFOs."""
                    t0 = j * TB
                    st = {}

                    def p_scores(b):
                        def f():
                            if b == 0:
                                st["psS"] = pa.tile([TB, BL, 128], DT, tag="s",
                                                    name="psS")
                            psS = st["psS"]
                            for k in range(KC):
                                nc.tensor.matmul(
                                    out=psS[:, b, :],
                                    lhsT=hd_dbt[:, k, b, t0:t0 + TB],
                                    rhs=hd_enc[:, k, b, :],
                                    start=(b == 0 and k == 0), stop=False)
                            nc.tensor.matmul(
                                out=psS[:, b, :], lhsT=ones1[:, :],
                                rhs=maddb[:, b, :],
                                start=False, stop=(b == BL - 1))
                        return f

                    def p_exp():
                        ex = awp.tile([TB, BL, 128], DT, tag="ex")
                        st["ex"] = ex
                        nc.scalar.activation(out=ex[:, :, :],
                                             in_=st["psS"][:, :, :],
                                             func=AF.Exp, scale=1.0 / 16.0)
                        st["alpha"] = awp.tile([TB, BL, 128], BF, tag="alpha",
                                               name="alpha")

                    def p_soft(b):
                        def f():
                            sm = wp.tile([TB, 1], DT, tag="sm")
                            nc.vector.reduce_sum(out=sm[:, :],
                                                 in_=st["ex"][:, b, :],
                                                 axis=AX.X)
                            rs = wp.tile([TB, 1], DT, tag="rs")
                            nc.vector.reciprocal(out=rs[:, :], in_=sm[:, :])
                            nc.vector.tensor_scalar(
                                out=st["alpha"][:, b, :], in0=st["ex"][:, b, :],
                                scalar1=rs[:, :1], scalar2=None, op0=ALU.mult)
                        return f

                    def p_aT(b):
                        def f():
                            if b == 0:
                                st["aT"] = awp.tile([128, BL, TB], BF,
                                                    tag="aT", name="aT")
                            psT = pa.tile([128, TB], DT, tag="t", name="psT")
                            nc.tensor.matmul(out=psT[:, :],
                                             lhsT=st["alpha"][:, b, :],
                                             rhs=identb[0:TB, 0:TB],
                                             start=True, stop=True)
                            nc.vector.tensor_copy(out=st["aT"][:, b, :],
                                                  in_=psT[:, :])
                        return f

                    def p_ctx(b, k):
                        def f():
                            if b == 0 and k == 0:
                                st["ctx"] = awp.tile([128, KC, TB, BL], BF,
                                                     tag="ctx", name="ctx")
                            psC = pa.tile([128, TB], DT, tag="c", name="psC")
                            nc.tensor.matmul(
                                out=psC[:, :], lhsT=ht_enc[:, b, k, :],
                                rhs=st["aT"][:, b, :], start=True, stop=True)
                            nc.vector.tensor_copy(
                                out=st["ctx"][:, k, :, b], in_=psC[:, :])
                        return f

                    def p_out(m):
                        def f():
                            psO = pa.tile([128, TB * BL], DT, tag="s",
                                          name="psO")
                            for k in range(KC):
                                nc.tensor.matmul(
                                    out=psO[:, :],
                                    lhsT=cwt_sb[:, k, m * 128:(m + 1) * 128],
                                    rhs=st["ctx"][:, k, :, :],
                                    start=(k == 0), stop=False)
                            nc.tensor.matmul(
                                out=psO[:, :], lhsT=identb[:, :],
                                rhs=hd_dec[:, m, t0 * BL:t0 * BL + TB * BL],
                                start=False, stop=True)
                            nc.vector.tensor_copy(
                                out=houts[:, m, j * 128:(j + 1) * 128],
                                in_=psO[:, :])
                            if m == KC - 1:
                                pending.extend((j, n0, nv)
                                               for (n0, nv) in dchunks)
                        return f

                    out = [p_scores(b) for b in range(BL)]
                    out.append(p_exp)
                    # per-b softmax->transpose->ctx so outproj unblocks early
                    for b in range(BL):
                        out.append(p_soft(b))
                        out.append(p_aT(b))
                        out.extend(p_ctx(b, k) for k in range(KC))
                    out.extend(p_out(m) for m in range(KC))
                    return out

                # projection: 1024-col double-chunks over four rotating
                # single-bank PSUM tiles; the two half-copies go to DVE and
                # ACT in parallel; one 1024-wide DMA per double-chunk.
                def emit_proj_chunk(j, n0, nv, engine):
                    lt = ltp.tile([128, 1024], BF, tag="lt", name="lt")
                    for h0 in range(0, nv, 512):
                        hv = min(512, nv - h0)
                        plt = pl.tile([128, 512], DT, tag="pl", name="plt")
                        for k in range(KC):
                            nc.tensor.matmul(
                                out=plt[:, :hv],
                                lhsT=houts[:, k, j * 128:(j + 1) * 128],
                                rhs=w_sb[:, k, n0 + h0:n0 + h0 + hv],
                                start=(k == 0), stop=(k == KC - 1))
                        if (h0 == 0) == (engine == "act"):
                            nc.scalar.copy(out=lt[:, h0:h0 + hv],
                                           in_=plt[:, :hv])
                        else:
                            nc.vector.tensor_copy(out=lt[:, h0:h0 + hv],
                                                  in_=plt[:, :hv])
                    nc.sync.dma_start(
                        out=out_d[j * 128:(j + 1) * 128, n0:n0 + nv],
                        in_=lt[:, :nv])

                for t in range(T):
                    bkd = pd.tile([128, KC, BL], DT, tag="d")
                    for m in range(KC):
                        for k in range(KC):
                            rhs = (hT_sb[:, k, :] if t == 0
                                   else hd_dec[:, k, (t - 1) * BL:t * BL])
                            nc.tensor.matmul(
                                out=bkd[:, m, :],
                                lhsT=u_sb[:, k, m * 128:(m + 1) * 128],
                                rhs=rhs, start=(m == 0 and k == 0), stop=False)
                    nc.tensor.matmul(
                        out=bkd[:, :, :], lhsT=identb[:, :],
                        rhs=xt[:, t, :, :], start=False, stop=True)
                    nc.scalar.activation(
                        out=hd_dec[:, :, t * BL:(t + 1) * BL], in_=bkd[:, :, :],
                        func=AF.Tanh)
                    nc.gpsimd.tensor_copy(out=hd_dbt[:, :, :, t],
                                          in_=hd_dec[:, :, t * BL:(t + 1) * BL])
                    # H' -> H^T transposes ride the idle first decoder steps
                    # (attention only needs ht_enc from step TB onwards)
                    if t < BL * KC:
                        b, k = divmod(t, KC)
                        tp2 = pa.tile([128, 128], DT, tag="s", name="tp2")
                        nc.tensor.matmul(
                            out=tp2[:, :], lhsT=hd_enc[:, k, b, :],
                            rhs=identb[:, :], start=True, stop=True)
                        nc.vector.tensor_copy(
                            out=ht_enc[:, b, k, :], in_=tp2[:, :])
                    if t % TB == TB - 1:
                        pieces.extend(make_attention_pieces(t // TB))
                        # kick the scores pieces immediately (they only
                        # need this block's now-complete hidden states)
                        for _ in range(4):
                            pieces.popleft()()
                    if interleave:
                        npop = 2 if len(pending) > 40 else 1
                        for i in range(npop):
                            if pending:
                                eng = ("act" if (t % 2 == 0 and i == 0)
                                       else "dve")
                                emit_proj_chunk(*pending.popleft(), engine=eng)
                    if pieces:
                        pieces.popleft()()
                # drain: finish last block's attention pieces + projection
                i = 0
                while pieces or pending:
                    if pending:
                        eng = "act" if i % 2 == 0 else "dve"
                        emit_proj_chunk(*pending.popleft(), engine=eng)
                        i += 1
                    if pieces:
                        pieces.popleft()()
    nc.compile()
    return nc


def _prep_in_maps(U, b_enc1, b_enc2, b_dec, E_en, E_de, ctx_W, W_out_de,
                  src_en, tgt_de_in):
    f32 = np.float32
    U = np.ascontiguousarray(U, f32).astype(NPBF)
    ctx_wt = np.ascontiguousarray(np.asarray(ctx_W, f32).T).astype(NPBF)
    w_out_t = np.ascontiguousarray(np.asarray(W_out_de, f32).T).astype(NPBF)
    E_en = (np.asarray(E_en, f32) + np.asarray(b_enc1, f32)[None, :]).astype(NPBF)
    E_de = (np.asarray(E_de, f32) + np.asarray(b_dec, f32)[None, :]).astype(NPBF)
    b2 = np.asarray(b_enc2, f32).reshape(KC, 128).T  # [128, KC]
    b2b = np.repeat(b2[:, :, None], BL, axis=2).reshape(128, KC * BL).astype(NPBF)
    src = np.asarray(src_en).astype(np.int32)
    tgt = np.asarray(tgt_de_in).astype(np.int32)
    in_maps = []
    for i in range(NCORES):
        b0 = i * BL
        in_maps.append({
            "u": U, "ctx_wt": ctx_wt, "w_out_t": w_out_t,
            "e_en": E_en, "e_de": E_de, "b2b": b2b,
            "src_idx": np.ascontiguousarray(src[:, b0:b0 + BL]),
            "tgt_idx": np.ascontiguousarray(tgt[:, b0:b0 + BL]),
        })
    return in_maps


def kernel(U, b_enc1, b_enc2, b_dec, E_en, E_de, ctx_W, W_out_de,
           src_en, tgt_de_in, _trace=False, _raw=False, _ncores=NCORES):
    if "nc" not in _CACHE:
        _CACHE["nc"] = _build()
    nc = _CACHE["nc"]
    in_maps = _prep_in_maps(U, b_enc1, b_enc2, b_dec, E_en, E_de, ctx_W,
                            W_out_de, src_en, tgt_de_in)[:_ncores]
    res = run_bass_kernel_spmd(nc, in_maps, list(range(_ncores)), trace=_trace)
    if _raw:
        return res
    logits = np.empty((T, _ncores * BL, V), np.float32)
    for i in range(_ncores):
        logits[:, i * BL:(i + 1) * BL, :] = (
            res.results[i]["out"].astype(np.float32).reshape(T, BL, V))
    if _trace:
        return logits, res
    return logits
